# revision 36
# baseline (speedup 1.0000x reference)
"""Trainium2 Bass kernel for NeuralGraphHidden (GNN message passing).

Full-input contract: kernel(**inputs) takes the complete unsharded arrays,
shards batch dim 0 across 8 NeuronCores (data parallel), runs one SPMD Bass
program, and reassembles the full output.

Math (per molecule, A=128 atoms, D=5 degree slots):
  deg[a]   = #(edges[a,:] != -1)
  out[a]   = relu(feat[a] @ W[deg[a]] + b[deg[a]])  if deg[a] < 5 else 0
  feat[a]  = [atoms[a] + sum_d atoms[edges[a,d]],  sum_d bonds[a,d]]

Degree-sparsity: deg[a] == 5 makes every mask (deg == d), d < 5, zero ->
the output row is structurally zero.  ~96% of atoms.  Of the rest, almost
all have deg == 4, so the kernel splits:
  main slots: deg == 4 atoms -> one dense with W[4]
  exc  slots: deg  < 4 atoms (a handful per core) -> masked multi-degree
              dense with the present degrees' weights

host (index metadata only, no float math): per core, molecules are
permuted so exception-bearing ones come first, then greedily packed into
bins of <=128 unique referenced atoms, <=24 main slots, <=4 exc slots
(exc only in the first 3 bins).  Gather/sum matrices ET[src_local, slot]
hold small integer counts.  Bond rows are gathered per slot.  All float
values flow through the device (bf16 operands / f32 accumulate); outputs
come back bf16 and are scattered into a zero f32 array.

device: featT = atoms_bin.T @ ET (TensorE), bond sums (VectorE),
Z = relu(featT.T @ [W4;b4]) for two 128-slot groups, masked per-degree
accumulated dense for exc slots.  DMAs are need-ordered across the three
queues (weights first); idle warm-up matmuls keep the PE HAM clock up.

A dense all-degrees fallback program is kept for the (never observed)
case that a capacity is exceeded.
"""

import sys

sys.path.insert(0, "/opt/trn_rl_repo")

import numpy as np

B, A, D = 256, 128, 5
FA, FB, C = 256, 64, 256
F = FA + FB        # 320
FAUG = F + 1       # 321 (bias row)
NCORES = 8
BL = B // NCORES   # 32 molecules per core

SLOTB = 26         # main slots per bin (deg==4 atoms)
EXB = 4            # exc slots per bin (deg<4 atoms)
EXBINS = 3         # only the first EXBINS bins may hold exc slots
EXCU = EXBINS * EXB  # 12 exc slots per core
NBMAX = 10         # bin-count cap (program is compiled per actual NB)

_CACHE = {}


# ---------------------------------------------------------------------------
# sparse program
# ---------------------------------------------------------------------------

def _build_program_sparse(L, NB):
    """L = #distinct exception degrees; NB = atom bins per core."""
    from contextlib import ExitStack

    import concourse.bass as bass
    import concourse.tile as tile
    from concourse import bacc, mybir

    f32 = mybir.dt.float32
    AF = mybir.ActivationFunctionType
    bf16 = mybir.dt.bfloat16

    MAIN = NB * SLOTB          # main slot columns (<=260)
    PF = NB * (SLOTB + EXB)    # psum gather columns per feature chunk
    LW = max(L, 1)
    LC = LW * C
    AUXC = D * EXCU + LW * EXCU
    MDO = D * EXCU             # aux col offset of the md masks
    assert 6 <= NB <= NBMAX

    nc = bacc.Bacc("TRN2", target_bir_lowering=False, debug=False,
                   num_devices=NCORES)

    # medium-size per-DMA tensors, each with per-partition-contiguous
    # layout (strided or sub-512B-per-partition transfers run at a
    # fraction of line rate)
    s1_d = nc.dram_tensor("s1", [A, PF + 2 * FA], bf16,
                          kind="ExternalInput")       # et | atoms b0 | b1
    s2_d = nc.dram_tensor("s2", [A, (NB - 4) * FA], bf16,
                          kind="ExternalInput")       # atoms b4..
    c1_d = nc.dram_tensor("c1", [A, 3 * C], bf16,
                          kind="ExternalInput")       # [W4;b4] 3 f-chunks
    c2_d = nc.dram_tensor("c2", [A, 2 * FA], bf16,
                          kind="ExternalInput")       # atoms b2 | b3
    bondm_d = nc.dram_tensor("bondm", [FB, D * MAIN], bf16,
                             kind="ExternalInput")
    aux_d = nc.dram_tensor("aux", [A, AUXC], bf16, kind="ExternalInput")
    wexc_d = nc.dram_tensor("wexc", [3, A, LC], bf16,
                            kind="ExternalInput")     # per f-chunk
    # main output, TRANSPOSED: out[r, ch*MAIN+s] = Z[slot s, C ch*128+r]
    out_d = nc.dram_tensor("out", [128, 2 * MAIN], bf16,
                           kind="ExternalOutput")
    oute_d = nc.dram_tensor("oute", [EXCU, C], bf16, kind="ExternalOutput")

    with tile.TileContext(nc) as tc, ExitStack() as ctx:
        consts = ctx.enter_context(tc.tile_pool(name="consts", bufs=1))
        pfeat = ctx.enter_context(tc.tile_pool(name="pfeat", bufs=1))
        pbs = ctx.enter_context(tc.tile_pool(name="pbs", bufs=2))
        pout = ctx.enter_context(tc.tile_pool(name="pout", bufs=3))
        ps_f = ctx.enter_context(
            tc.tile_pool(name="ps_f", bufs=1, space="PSUM"))
        ps_z = ctx.enter_context(
            tc.tile_pool(name="ps_z", bufs=2, space="PSUM"))
        ps_e = ctx.enter_context(
            tc.tile_pool(name="ps_e", bufs=1, space="PSUM"))

        t_s1 = consts.tile([A, PF + 2 * FA], bf16)
        t_s2 = consts.tile([A, (NB - 4) * FA], bf16)
        t_c1 = consts.tile([A, 3 * C], bf16)
        t_c2 = consts.tile([A, 2 * FA], bf16)
        bondm = pbs.tile([FB, D * MAIN], bf16)
        aux = consts.tile([A, AUXC], bf16)
        wexc = [consts.tile([A, LC], bf16, name=f"wexc{i}")
                for i in range(3)]

        # ---- input DMAs, need-ordered per queue: atoms first, then
        # weights/bonds, exception weights last ----------------------------
        nc.sync.dma_start(out=t_s1[:], in_=s1_d.ap()[:])
        nc.sync.dma_start(out=t_s2[:], in_=s2_d.ap()[:])
        nc.sync.dma_start(out=wexc[2][:], in_=wexc_d.ap()[2])
        nc.scalar.dma_start(out=t_c2[:], in_=c2_d.ap()[:])
        nc.scalar.dma_start(out=t_c1[:, 0:384], in_=c1_d.ap()[:, 0:384])
        nc.scalar.dma_start(out=t_c1[:, 384:768],
                            in_=c1_d.ap()[:, 384:768])
        nc.scalar.dma_start(out=wexc[1][:], in_=wexc_d.ap()[1])
        nc.gpsimd.dma_start(out=bondm[:], in_=bondm_d.ap()[:])
        nc.gpsimd.dma_start(out=aux[:], in_=aux_d.ap()[:])
        nc.gpsimd.dma_start(out=wexc[0][:], in_=wexc_d.ap()[0])

        def atile(b):
            if b < 2:
                return t_s1[:, PF + b * FA:PF + (b + 1) * FA]
            if b < 4:
                return t_c2[:, (b - 2) * FA:(b - 1) * FA]
            return t_s2[:, (b - 4) * FA:(b - 3) * FA]

        # ---- bond sums -> feature chunk2 ([bond rows; ones row]) ----------
        chunk2m = pfeat.tile([FAUG - 256, MAIN], bf16)
        nc.vector.memset(chunk2m[FB:FB + 1, :], 1.0)
        chunk2e = pfeat.tile([FAUG - 256, EXCU], bf16)
        nc.vector.memset(chunk2e[FB:FB + 1, :], 1.0)

        with nc.allow_low_precision(reason="bf16 bond sums, rel ~4e-3"):
            t01 = pbs.tile([FB, MAIN], bf16, tag="bt01")
            t23 = pbs.tile([FB, MAIN], bf16, tag="bt23")
            nc.vector.tensor_add(t01[:], bondm[:, 0:MAIN],
                                 bondm[:, MAIN:2 * MAIN])
            nc.vector.tensor_add(t23[:], bondm[:, 2 * MAIN:3 * MAIN],
                                 bondm[:, 3 * MAIN:4 * MAIN])
            nc.vector.tensor_add(t01[:], t01[:], t23[:])
            nc.vector.tensor_add(chunk2m[0:FB, :], t01[:],
                                 bondm[:, 4 * MAIN:5 * MAIN])
            e01 = pbs.tile([FB, EXCU], bf16, tag="be01")
            e23 = pbs.tile([FB, EXCU], bf16, tag="be23")
            nc.gpsimd.tensor_add(e01[:], aux[0:FB, 0:EXCU],
                                 aux[0:FB, EXCU:2 * EXCU])
            nc.gpsimd.tensor_add(e23[:], aux[0:FB, 2 * EXCU:3 * EXCU],
                                 aux[0:FB, 3 * EXCU:4 * EXCU])
            nc.gpsimd.tensor_add(e01[:], e01[:], e23[:])
            nc.gpsimd.tensor_add(chunk2e[0:FB, :], e01[:],
                                 aux[0:FB, 4 * EXCU:5 * EXCU])

        c2em = [pfeat.tile([FAUG - 256, EXCU], bf16, name=f"c2em{i}")
                for i in range(L)]
        with nc.allow_low_precision(reason="exact 0/1 masking"):
            for i in range(L):
                nc.gpsimd.tensor_mul(
                    c2em[i][:], chunk2e[:],
                    aux[0:FAUG - 256,
                        MDO + i * EXCU:MDO + (i + 1) * EXCU])

        # ---- neighbour+self gather: featT = atoms_bin.T @ ET --------------
        KB = SLOTB + EXB
        pf0 = ps_f.tile([A, PF], f32, tag="pf0")
        pf1 = ps_f.tile([A, PF], f32, tag="pf1")

        def gather(b):
            at = atile(b)
            ecols = t_s1[:, b * KB:(b + 1) * KB]
            nc.tensor.matmul(pf0[:, b * KB:(b + 1) * KB],
                             at[:, 0:128], ecols)
            nc.tensor.matmul(pf1[:, b * KB:(b + 1) * KB],
                             at[:, 128:256], ecols)

        # expected arrival order: b0,b1 (sync 1st), b2,b3 (scalar 1st),
        # b6.. (gpsimd 1st), b4,b5 (sync 2nd)
        gather(0)
        gather(1)
        gather(2)
        gather(3)
        for b in range(6, NB):
            gather(b)
        gather(4)
        gather(5)

        # compacted transposed features: main slots then exc slots
        featm0 = pfeat.tile([128, MAIN], bf16)
        featm1 = pfeat.tile([128, MAIN], bf16)
        fe0m = [pfeat.tile([128, EXCU], bf16, name=f"fe0m{i}")
                for i in range(L)]
        fe1m = [pfeat.tile([128, EXCU], bf16, name=f"fe1m{i}")
                for i in range(L)]
        with nc.allow_low_precision(reason="bf16 features, rel ~4e-3"):
            nc.scalar.copy(
                featm0.rearrange("p (j k) -> p j k", j=NB),
                pf0.rearrange("p (j k) -> p j k", j=NB)[:, :, 0:SLOTB])
            nc.vector.tensor_copy(
                featm1.rearrange("p (j k) -> p j k", j=NB),
                pf1.rearrange("p (j k) -> p j k", j=NB)[:, :, 0:SLOTB])
            for i in range(L):
                mcols = aux[:, MDO + i * EXCU:
                            MDO + (i + 1) * EXCU].rearrange(
                    "p (j k) -> p j k", j=EXBINS)
                nc.vector.tensor_mul(
                    fe0m[i].rearrange("p (j k) -> p j k", j=EXBINS),
                    pf0.rearrange("p (j k) -> p j k",
                                  j=NB)[:, 0:EXBINS, SLOTB:KB], mcols)
                nc.vector.tensor_mul(
                    fe1m[i].rearrange("p (j k) -> p j k", j=EXBINS),
                    pf1.rearrange("p (j k) -> p j k",
                                  j=NB)[:, 0:EXBINS, SLOTB:KB], mcols)

        # ---- main dense, weights stationary (transposed output):
        #      ZT[ch] = relu(W4[:, ch].T @ feat)  [128 C-rows, MAIN slots]
        outt = pout.tile([128, 2 * MAIN], bf16)
        for ch in range(2):
            co = ch * 384
            pz = ps_z.tile([128, MAIN], f32, tag=f"pzT{ch}")
            nc.tensor.matmul(pz[:], t_c1[:, co:co + 128], featm0[:],
                             start=True, stop=False)
            nc.tensor.matmul(pz[:], t_c1[:, co + 128:co + 256], featm1[:],
                             start=False, stop=False)
            nc.tensor.matmul(pz[:], t_c1[0:FAUG - 256, co + 256:co + 384],
                             chunk2m[:], start=False, stop=True)
            nc.scalar.activation(outt[:, ch * MAIN:(ch + 1) * MAIN], pz[:],
                                 AF.Relu)
        nc.sync.dma_start(out=out_d.ap()[:], in_=outt[:])

        # ---- exception slots: masked per-degree dense ---------------------
        if L > 0:
            # f-chunk-major so each group runs as its wexc chunk lands
            pze = ps_e.tile([EXCU, C], f32, tag="pze")
            for i in range(L):
                nc.tensor.matmul(pze[:], fe0m[i][:],
                                 wexc[0][:, i * C:(i + 1) * C],
                                 start=(i == 0), stop=False)
            for i in range(L):
                nc.tensor.matmul(pze[:], fe1m[i][:],
                                 wexc[1][:, i * C:(i + 1) * C],
                                 start=False, stop=False)
            for i in range(L):
                nc.tensor.matmul(pze[:], c2em[i][:],
                                 wexc[2][0:FAUG - 256, i * C:(i + 1) * C],
                                 start=False, stop=(i == L - 1))
            oute = pout.tile([EXCU, C], bf16)
            nc.scalar.activation(oute[:], pze[:], AF.Relu)
            nc.scalar.dma_start(out=oute_d.ap()[:], in_=oute[:])

    nc.compile()
    return nc


def _sparse_metadata(edges):
    """Host-side index metadata: bin packing + gather/sum matrices.

    Returns None if any capacity is exceeded (caller falls back to the
    dense program).
    """
    import ml_dtypes

    bf = ml_dtypes.bfloat16
    deg = (edges != -1).sum(axis=2)                      # (B, A)
    main_mask = deg == D - 1                             # deg == 4
    exc_mask = deg < D - 1                               # deg <= 3
    KB = SLOTB + EXB

    # per-core first-fit-decreasing packing; exception-bearing molecules
    # first-fit into bins 0..EXBINS-1 only
    per_core = []
    nb_max = 0
    for c in range(NCORES):
        stats = []
        for m in range(BL):
            bm = c * BL + m
            sel = np.nonzero(main_mask[bm])[0]
            exc = np.nonzero(exc_mask[bm])[0]
            if len(sel) + len(exc) == 0:
                continue
            uniq = set()
            for a in list(sel) + list(exc):
                uniq.add(int(a))
                for e in edges[bm, a]:
                    if e >= 0:
                        uniq.add(int(e))
            if len(uniq) > A or len(sel) > SLOTB or len(exc) > EXB:
                return None
            stats.append((m, len(uniq), len(sel), len(exc)))
        bins = []          # list of [rows, slots, excs, mols]
        ok = [True]

        def fit(m, nr, nsl, nex, hi):
            for bi in range(min(hi, len(bins))):
                r, s, e, mols = bins[bi]
                if r + nr <= A and s + nsl <= SLOTB and e + nex <= EXB:
                    bins[bi] = [r + nr, s + nsl, e + nex, mols + [m]]
                    return
            if len(bins) < hi:
                bins.append([nr, nsl, nex, [m]])
                return
            ok[0] = False

        for m, nr, nsl, nex in [t for t in stats if t[3] > 0]:
            fit(m, nr, nsl, nex, EXBINS)
        for m, nr, nsl, nex in sorted([t for t in stats if t[3] == 0],
                                      key=lambda t: -t[1]):
            fit(m, nr, nsl, nex, NBMAX)
        if not ok[0]:
            return None
        per_core.append([b[3] for b in bins])
        nb_max = max(nb_max, len(bins))
    NB = nb_max
    MAIN = NB * SLOTB

    et = np.zeros((NCORES, A, NB * KB), dtype=np.float32)
    gidx = np.zeros((NCORES, NB, A), dtype=np.int64)
    main_rows = [[] for _ in range(NCORES)]   # (slot, mol, atom)
    exc_rows = [[] for _ in range(NCORES)]    # (eslot, mol, atom)
    bidx_m = np.zeros((NCORES, MAIN), dtype=np.int64)
    bval_m = np.zeros((NCORES, MAIN), dtype=bool)
    bidx_e = np.zeros((NCORES, EXCU), dtype=np.int64)
    bval_e = np.zeros((NCORES, EXCU), dtype=bool)
    excdeg = {}

    for c in range(NCORES):
        for bnum, mols in enumerate(per_core[c]):
            off = 0            # bin-local gathered-atom row
            k = 0              # main slot within bin
            ke = 0             # exc slot within bin
            loc = {}
            uniq = []
            for m in mols:
                bm = c * BL + m
                sel = np.nonzero(main_mask[bm])[0]
                exc = np.nonzero(exc_mask[bm])[0]
                for a in list(sel) + list(exc):
                    srcs = [int(a)] + [int(e) for e in edges[bm, a] if e >= 0]
                    for s2 in srcs:
                        key = (m, s2)
                        if key not in loc:
                            loc[key] = len(uniq)
                            uniq.append(key)
                    if deg[bm, a] == D - 1:
                        col = bnum * KB + k
                        slot = bnum * SLOTB + k
                        k += 1
                        main_rows[c].append((slot, m, int(a)))
                        bidx_m[c, slot] = m * A + int(a)
                        bval_m[c, slot] = True
                    else:
                        col = bnum * KB + SLOTB + ke
                        eslot = bnum * EXB + ke
                        ke += 1
                        exc_rows[c].append((eslot, m, int(a)))
                        bidx_e[c, eslot] = m * A + int(a)
                        bval_e[c, eslot] = True
                        excdeg[(c, eslot)] = int(deg[bm, a])
                    for s2 in srcs:
                        et[c, loc[(m, s2)], col] += 1.0
            assert len(uniq) <= A
            for r, (m, s2) in enumerate(uniq):
                gidx[c, bnum, r] = m * A + s2

    dlist = sorted(set(excdeg.values()))
    L = len(dlist)
    LW = max(L, 1)
    md = np.zeros((NCORES, A, LW * EXCU), dtype=np.float32)
    for (c, eslot), d in excdeg.items():
        md[c, :, dlist.index(d) * EXCU + eslot] = 1.0
    return {
        "NB": NB,
        "dlist": dlist,
        "et": et.astype(bf),
        "md": md.astype(bf),
        "gidx": gidx,
        "main_rows": main_rows,
        "exc_rows": exc_rows,
        "bidx_m": bidx_m,
        "bval_m": bval_m,
        "bidx_e": bidx_e,
        "bval_e": bval_e,
    }


def _make_in_maps_sparse(atoms, bonds, W, b, meta):
    import ml_dtypes

    bf = ml_dtypes.bfloat16
    NB = meta["NB"]
    MAIN = NB * SLOTB
    PF = NB * (SLOTB + EXB)
    L = len(meta["dlist"])
    LW = max(L, 1)
    LC = LW * C
    AUXC = D * EXCU + LW * EXCU

    atoms_flat = atoms.reshape(NCORES, BL * A, FA)
    # gathered atoms, partition-major: [NCORES, A, NB, FA]
    atoms8 = atoms_flat[np.arange(NCORES)[:, None, None],
                        meta["gidx"]].transpose(0, 2, 1, 3)

    waug = np.concatenate([W, b[:, None, :]], axis=1)     # (5, 321, 256)
    # ch-major: [ch half][f chunk] of 128 cols each
    wd4p = np.zeros((128, 3 * C), dtype=np.float32)
    for ch in range(2):
        cs = ch * 128
        wd4p[:, ch * 384:ch * 384 + 128] = waug[D - 1][0:128, cs:cs + 128]
        wd4p[:, ch * 384 + 128:ch * 384 + 256] = \
            waug[D - 1][128:256, cs:cs + 128]
        wd4p[0:FAUG - 256, ch * 384 + 256:ch * 384 + 384] = \
            waug[D - 1][256:FAUG, cs:cs + 128]
    wexc = np.zeros((3, 128, LC), dtype=np.float32)       # per f-chunk
    for i, d in enumerate(meta["dlist"]):
        wexc[0, :, i * C:(i + 1) * C] = waug[d][0:128]
        wexc[1, :, i * C:(i + 1) * C] = waug[d][128:256]
        wexc[2, 0:FAUG - 256, i * C:(i + 1) * C] = waug[d][256:FAUG]
    wd4p = wd4p.astype(bf)
    wexc = wexc.astype(bf)

    bonds_flat = bonds.reshape(NCORES, BL * A, D, FB)
    in_maps = []
    for c in range(NCORES):
        bm = bonds_flat[c][meta["bidx_m"][c]]             # (MAIN, D, FB)
        bm = bm * meta["bval_m"][c][:, None, None]
        be = bonds_flat[c][meta["bidx_e"][c]]             # (EXCU, D, FB)
        be = be * meta["bval_e"][c][:, None, None]
        bondm = np.ascontiguousarray(
            bm.transpose(2, 1, 0).reshape(FB, D * MAIN)).astype(bf)
        auxp = np.zeros((A, AUXC), dtype=np.float32)
        auxp[0:FB, 0:D * EXCU] = be.transpose(2, 1, 0).reshape(FB, D * EXCU)
        auxp[:, D * EXCU:AUXC] = meta["md"][c]

        s1 = np.zeros((A, PF + 2 * FA), dtype=np.float32)
        s1[:, 0:PF] = meta["et"][c]
        s1[:, PF:PF + FA] = atoms8[c, :, 0]
        s1[:, PF + FA:PF + 2 * FA] = atoms8[c, :, 1]
        s2 = atoms8[c, :, 4:NB].reshape(A, (NB - 4) * FA)
        c2 = np.concatenate([atoms8[c, :, 2], atoms8[c, :, 3]], axis=1)
        im = {
            "s1": s1.astype(bf),
            "s2": np.ascontiguousarray(s2).astype(bf),  # bins 4..
            "c1": wd4p,
            "c2": np.ascontiguousarray(c2).astype(bf),
            "bondm": bondm,
            "aux": auxp.astype(bf),
            "wexc": wexc,
        }
        in_maps.append(im)
    return in_maps


# ---------------------------------------------------------------------------
# dense fallback program (unchanged from the baseline kernel)
# ---------------------------------------------------------------------------

def _build_program_dense():
    from contextlib import ExitStack

    import concourse.bass as bass
    import concourse.tile as tile
    from concourse import bacc, mybir

    f32 = mybir.dt.float32
    AF = mybir.ActivationFunctionType
    OP = mybir.AluOpType
    f32r = mybir.dt.float32r
    bf16 = mybir.dt.bfloat16

    nc = bacc.Bacc("TRN2", target_bir_lowering=False, debug=False,
                   num_devices=NCORES)

    atoms_d = nc.dram_tensor("atoms", [BL, A, FA], f32r, kind="ExternalInput")
    bonds_d = nc.dram_tensor("bonds", [BL, A, D * FB], f32,
                             kind="ExternalInput")
    edges_d = nc.dram_tensor("edges", [BL, A, A * D], bf16,
                             kind="ExternalInput")
    waug_d = nc.dram_tensor("waug", [D, FAUG, C], f32r, kind="ExternalInput")
    ident_d = nc.dram_tensor("ident", [A, A], f32, kind="ExternalInput")
    identr_d = nc.dram_tensor("identr", [A, A], f32r, kind="ExternalInput")
    identb_d = nc.dram_tensor("identb", [A, A], bf16, kind="ExternalInput")
    iota_d = nc.dram_tensor("iota", [A, 1], f32, kind="ExternalInput")
    edeg_d = nc.dram_tensor("edeg", [BL, A, D], f32, kind="ExternalInput")
    onesr_d = nc.dram_tensor("onesr", [1, A], f32, kind="ExternalInput")
    out_d = nc.dram_tensor("out", [BL, A, C], f32, kind="ExternalOutput")

    atoms_ap = atoms_d.ap()
    bonds_ap = bonds_d.ap()
    edges_ap = edges_d.ap()
    out_ap = out_d.ap()

    with tile.TileContext(nc) as tc, ExitStack() as ctx:
        consts = ctx.enter_context(tc.tile_pool(name="consts", bufs=1))
        pin = ctx.enter_context(tc.tile_pool(name="pin", bufs=3))
        pbc = ctx.enter_context(tc.tile_pool(name="pbc", bufs=2))
        pet = ctx.enter_context(tc.tile_pool(name="pet", bufs=2))
        pfeat = ctx.enter_context(tc.tile_pool(name="pfeat", bufs=2))
        pmd = ctx.enter_context(tc.tile_pool(name="pmd", bufs=2))
        pz = ctx.enter_context(tc.tile_pool(name="pz", bufs=2))
        pout = ctx.enter_context(tc.tile_pool(name="pout", bufs=3))
        ps_f = ctx.enter_context(
            tc.tile_pool(name="ps_f", bufs=2, space="PSUM"))
        ps_c2 = ctx.enter_context(
            tc.tile_pool(name="ps_c2", bufs=1, space="PSUM"))
        ps_z = ctx.enter_context(
            tc.tile_pool(name="ps_z", bufs=1, space="PSUM"))
        ps_s = ctx.enter_context(
            tc.tile_pool(name="ps_s", bufs=1, space="PSUM"))

        G4 = 4
        ident = consts.tile([A, A], f32)
        nc.scalar.dma_start(out=ident[:], in_=ident_d.ap()[:])
        identr = consts.tile([A, A], f32r)
        nc.scalar.dma_start(out=identr[:], in_=identr_d.ap()[:])
        iota_col = consts.tile([A, 1], f32)
        nc.gpsimd.dma_start(out=iota_col[:], in_=iota_d.ap()[:])
        ones_row = consts.tile([1, A], f32)
        nc.scalar.dma_start(out=ones_row[:], in_=onesr_d.ap()[:])
        identb4 = consts.tile([A, G4 * A], bf16)
        for j in range(G4):
            nc.gpsimd.dma_start(out=identb4[:, j * A:(j + 1) * A],
                                in_=identb_d.ap()[:])

        w0 = consts.tile([128, D * C], f32r)
        w1 = consts.tile([128, D * C], f32r)
        w2 = consts.tile([FAUG - 256, D * C], f32r)
        for d in range(D):
            nc.scalar.dma_start(out=w0[:, d * C:(d + 1) * C],
                                in_=waug_d.ap()[d, 0:128, :])
            nc.scalar.dma_start(out=w1[:, d * C:(d + 1) * C],
                                in_=waug_d.ap()[d, 128:256, :])
            nc.scalar.dma_start(out=w2[:, d * C:(d + 1) * C],
                                in_=waug_d.ap()[d, 256:FAUG, :])

        for bg in range(BL // G4):
            mols = range(bg * G4, (bg + 1) * G4)
            atoms4 = pin.tile([A, G4 * FA], f32r)
            nc.sync.dma_start(
                out=atoms4.rearrange("p (g f) -> p g f", g=G4),
                in_=atoms_ap[bg * G4:(bg + 1) * G4].rearrange(
                    "g p f -> p g f"))
            bonds4 = pin.tile([A, G4 * D * FB], f32)
            nc.sync.dma_start(
                out=bonds4.rearrange("p (g f) -> p g f", g=G4),
                in_=bonds_ap[bg * G4:(bg + 1) * G4].rearrange(
                    "g p f -> p g f"))
            bc_e4 = pbc.tile([A, G4 * A * D], bf16)
            nc.gpsimd.dma_start(
                out=bc_e4.rearrange("p (g f) -> p g f", g=G4),
                in_=edges_ap[bg * G4:(bg + 1) * G4].rearrange(
                    "g p f -> p g f"))
            edeg4 = pfeat.tile([A, G4 * D], f32)
            nc.sync.dma_start(
                out=edeg4.rearrange("p (g f) -> p g f", g=G4),
                in_=edeg_d.ap()[bg * G4:(bg + 1) * G4].rearrange(
                    "g p f -> p g f"))
            ne4 = pfeat.tile([A, G4 * D], f32)
            nc.vector.tensor_scalar(ne4[:], edeg4[:], -1.0, None,
                                    OP.not_equal)
            degp1_4 = pfeat.tile([A, G4], f32)
            nc.vector.tensor_reduce(
                degp1_4[:], ne4.rearrange("p (g d) -> p g d", g=G4),
                axis=mybir.AxisListType.X, op=OP.add)
            nc.vector.tensor_scalar(degp1_4[:], degp1_4[:], 1.0, None,
                                    OP.add)

            cmp5 = pbc.tile([A, G4 * A * D], bf16)
            nc.vector.tensor_scalar(cmp5[:], bc_e4[:], iota_col[:], None,
                                    OP.is_equal)
            cg = cmp5.rearrange("p (g d a) -> p g d a", g=G4, d=D)
            t01 = pet.tile([A, G4 * A], bf16)
            nc.vector.tensor_add(t01[:], cg[:, :, 0, :], cg[:, :, 1, :])
            t23 = pet.tile([A, G4 * A], bf16)
            nc.vector.tensor_add(t23[:], cg[:, :, 2, :], cg[:, :, 3, :])
            t4i = pet.tile([A, G4 * A], bf16)
            nc.vector.tensor_add(t4i[:], cg[:, :, 4, :], identb4[:])
            t0123 = pet.tile([A, G4 * A], bf16)
            nc.vector.tensor_add(t0123[:], t01[:], t23[:])
            etp4 = pet.tile([A, G4 * A], f32r)
            with nc.allow_low_precision(reason="exact small-int counts"):
                nc.vector.tensor_add(etp4[:], t0123[:], t4i[:])

            out4 = pout.tile([A, G4 * C], f32)
            for j, bm in enumerate(mols):
                etp = etp4[:, j * A:(j + 1) * A]
                atoms_sb = atoms4[:, j * FA:(j + 1) * FA]
                bonds_sb = bonds4[:, j * D * FB:(j + 1) * D * FB]

                degp1 = degp1_4[:, j:j + 1]

                pf = ps_f.tile([A, FA], f32)
                nc.tensor.matmul(pf[:, 0:128], atoms_sb[:, 0:128], etp)
                nc.tensor.matmul(pf[:, 128:256], atoms_sb[:, 128:256], etp)

                featT01 = pfeat.tile([A, FA], f32r)
                nc.scalar.copy(featT01[:], pf[:, 0:FA])

                sumbond = pfeat.tile([A, FB], f32r)
                with nc.allow_low_precision(
                        reason="f32r rounding of bond sums"):
                    nc.vector.reduce_sum(
                        sumbond[:],
                        bonds_sb.rearrange("p (d f) -> p f d", d=D),
                        axis=mybir.AxisListType.X)
                pc2 = ps_c2.tile([FB, A], f32)
                nc.tensor.matmul(pc2[:], sumbond[:], identr[:])
                chunk2 = pfeat.tile([FAUG - 256, A], f32r)
                nc.scalar.copy(chunk2[0:FB, :], pc2[:])
                nc.vector.tensor_copy(chunk2[FB:FB + 1, :], ones_row[:])

                md = pmd.tile([A, D * A], f32r)
                for d in range(D):
                    nc.vector.tensor_scalar(md[:, d * A:(d + 1) * A],
                                            ident[:], degp1[:], float(d + 1),
                                            OP.mult, OP.is_equal)

                lhs = [featT01[:, 0:128], featT01[:, 128:256], chunk2[:]]
                rhs = [w0, w1, w2]
                groups = [(0, 512), (512, 1024), (1024, 1280)]
                zsb = pz.tile([A, D * C], f32r)
                for g0, g1 in groups:
                    pzg = ps_z.tile([A, 512], f32, tag="pzg", bufs=4)
                    nc.tensor.matmul(pzg[:, 0:g1 - g0], lhs[0],
                                     rhs[0][:, g0:g1], start=True, stop=False)
                    nc.tensor.matmul(pzg[:, 0:g1 - g0], lhs[1],
                                     rhs[1][:, g0:g1], start=False,
                                     stop=False)
                    nc.tensor.matmul(pzg[:, 0:g1 - g0], lhs[2],
                                     rhs[2][:, g0:g1], start=False, stop=True)
                    nc.scalar.copy(zsb[:, g0:g1], pzg[:, 0:g1 - g0])

                pst = ps_s.tile([A, C], f32)
                for d in range(D):
                    nc.tensor.matmul(pst[:], md[:, d * A:(d + 1) * A],
                                     zsb[:, d * C:(d + 1) * C],
                                     start=(d == 0), stop=(d == D - 1))
                nc.scalar.activation(out4[:, j * C:(j + 1) * C], pst[:],
                                     AF.Relu)
            nc.gpsimd.dma_start(
                out=out_ap[bg * G4:(bg + 1) * G4].rearrange("g p f -> p g f"),
                in_=out4.rearrange("p (g f) -> p g f", g=G4))

    nc.compile()
    return nc


def _make_in_maps_dense(atoms, bonds, edges, W, b):
    atoms = np.ascontiguousarray(np.asarray(atoms, dtype=np.float32))
    bonds = np.ascontiguousarray(np.asarray(bonds, dtype=np.float32))
    edges = np.asarray(edges)
    W = np.asarray(W, dtype=np.float32)
    b = np.asarray(b, dtype=np.float32)

    import ml_dtypes
    edges_f = np.ascontiguousarray(edges.transpose(0, 2, 1)).reshape(
        B, D * A).astype(ml_dtypes.bfloat16)
    edges_rep = np.ascontiguousarray(
        np.broadcast_to(edges_f[:, None, :], (B, A, D * A)))

    waug = np.ascontiguousarray(
        np.concatenate([W, b[:, None, :]], axis=1))           # (5, 321, 256)
    ident = np.eye(A, dtype=np.float32)
    iota = np.arange(A, dtype=np.float32).reshape(A, 1)
    onesr = np.ones((1, A), dtype=np.float32)

    edeg8 = edges.reshape(NCORES, BL, A, D).astype(np.float32)
    atoms8 = atoms.reshape(NCORES, BL, A, FA)
    bonds8 = bonds.reshape(NCORES, BL, A, D * FB)
    edges8 = edges_rep.reshape(NCORES, BL, A, A * D)

    return [
        {
            "atoms": atoms8[c],
            "bonds": bonds8[c],
            "edges": edges8[c],
            "waug": waug,
            "ident": ident,
            "identr": ident,
            "identb": ident.astype(ml_dtypes.bfloat16),
            "iota": iota,
            "edeg": edeg8[c],
            "onesr": onesr,
        }
        for c in range(NCORES)
    ]


# ---------------------------------------------------------------------------
# entry points
# ---------------------------------------------------------------------------

def run_sharded(atoms, bonds, edges, W, b, trace=False):
    """Run on the 8 NeuronCores; returns (output, BassKernelResults)."""
    from concourse.bass_utils import run_bass_kernel_spmd

    atoms = np.ascontiguousarray(np.asarray(atoms, dtype=np.float32))
    bonds = np.ascontiguousarray(np.asarray(bonds, dtype=np.float32))
    edges = np.asarray(edges)
    W = np.asarray(W, dtype=np.float32)
    b = np.asarray(b, dtype=np.float32)

    meta = _sparse_metadata(edges)
    if meta is None:
        if "dense" not in _CACHE:
            _CACHE["dense"] = _build_program_dense()
        nc = _CACHE["dense"]
        in_maps = _make_in_maps_dense(atoms, bonds, edges, W, b)
        res = run_bass_kernel_spmd(nc, in_maps, list(range(NCORES)),
                                   trace=trace)
        out = np.concatenate(
            [res.results[c]["out"] for c in range(NCORES)],
            axis=0).reshape(B, A, C)
        return out, res

    L = len(meta["dlist"])
    NB = meta["NB"]
    key = ("sparse", L, NB)
    if key not in _CACHE:
        _CACHE[key] = _build_program_sparse(L, NB)
    nc = _CACHE[key]
    in_maps = _make_in_maps_sparse(atoms, bonds, W, b, meta)
    res = run_bass_kernel_spmd(nc, in_maps, list(range(NCORES)), trace=trace)

    MAIN = NB * SLOTB
    out = np.zeros((B, A, C), dtype=np.float32)
    for c in range(NCORES):
        # out is transposed on device: [128, 2*MAIN], C row = ch*128+r
        arr = np.asarray(res.results[c]["out"], dtype=np.float32)
        zt = np.concatenate([arr[:, 0:MAIN], arr[:, MAIN:2 * MAIN]], axis=0)
        main = zt.T                                      # [MAIN, C]
        exc = np.asarray(res.results[c]["oute"], dtype=np.float32)
        for slot, ml, a in meta["main_rows"][c]:
            out[c * BL + ml, a] = main[slot]
        for eslot, ml, a in meta["exc_rows"][c]:
            out[c * BL + ml, a] = exc[eslot]
    return out, res


def kernel(atoms, bonds, edges, W, b):
    out, _ = run_sharded(atoms, bonds, edges, W, b)
    return out


# revision 37
# speedup vs baseline: 1.1377x; 1.1377x over previous
"""Trainium2 Bass kernel for NeuralGraphHidden (GNN message passing).

Full-input contract: kernel(**inputs) takes the complete unsharded arrays,
shards batch dim 0 across 8 NeuronCores (data parallel), runs one SPMD Bass
program, and reassembles the full output.

Math (per molecule, A=128 atoms, D=5 degree slots):
  deg[a]   = #(edges[a,:] != -1)
  out[a]   = relu(feat[a] @ W[deg[a]] + b[deg[a]])  if deg[a] < 5 else 0
  feat[a]  = [atoms[a] + sum_d atoms[edges[a,d]],  sum_d bonds[a,d]]

Degree-sparsity: deg[a] == 5 makes every mask (deg == d), d < 5, zero ->
the output row is structurally zero.  ~96% of atoms.  Of the rest, almost
all have deg == 4, so the kernel splits:
  main slots: deg == 4 atoms -> one dense with W[4]
  exc  slots: deg  < 4 atoms (a handful per core) -> masked multi-degree
              dense with the present degrees' weights

host (index metadata only, no float math): per core, molecules are
permuted so exception-bearing ones come first, then greedily packed into
bins of <=128 unique referenced atoms, <=24 main slots, <=4 exc slots
(exc only in the first 3 bins).  Gather/sum matrices ET[src_local, slot]
hold small integer counts.  Bond rows are gathered per slot.  All float
values flow through the device (bf16 operands / f32 accumulate); outputs
come back bf16 and are scattered into a zero f32 array.

device: featT = atoms_bin.T @ ET (TensorE), bond sums (VectorE),
Z = relu(featT.T @ [W4;b4]) for two 128-slot groups, masked per-degree
accumulated dense for exc slots.  DMAs are need-ordered across the three
queues (weights first); idle warm-up matmuls keep the PE HAM clock up.

A dense all-degrees fallback program is kept for the (never observed)
case that a capacity is exceeded.
"""

import sys

sys.path.insert(0, "/opt/trn_rl_repo")

import numpy as np

B, A, D = 256, 128, 5
FA, FB, C = 256, 64, 256
F = FA + FB        # 320
FAUG = F + 1       # 321 (bias row)
NCORES = 8
BL = B // NCORES   # 32 molecules per core

SLOTB = 26         # main slots per bin (deg==4 atoms)
EXB = 4            # exc slots per bin (deg<4 atoms)
EXBINS = 3         # only the first EXBINS bins may hold exc slots
EXCU = EXBINS * EXB  # 12 exc slots per core
NBMAX = 10         # bin-count cap (program is compiled per actual NB)

_CACHE = {}


# ---------------------------------------------------------------------------
# sparse program
# ---------------------------------------------------------------------------

def _build_program_sparse(L, NB):
    """L = #distinct exception degrees; NB = atom bins per core."""
    from contextlib import ExitStack

    import concourse.bass as bass
    import concourse.tile as tile
    from concourse import bacc, mybir

    f32 = mybir.dt.float32
    AF = mybir.ActivationFunctionType
    bf16 = mybir.dt.bfloat16

    MAIN = NB * SLOTB          # main slot columns (<=260)
    PF = NB * (SLOTB + EXB)    # psum gather columns per feature chunk
    LW = max(L, 1)
    LC = LW * C
    AUXC = D * EXCU + LW * EXCU
    MDO = D * EXCU             # aux col offset of the md masks
    assert 6 <= NB <= NBMAX

    nc = bacc.Bacc("TRN2", target_bir_lowering=False, debug=False,
                   num_devices=NCORES)

    # medium-size per-DMA tensors, each with per-partition-contiguous
    # layout (strided or sub-512B-per-partition transfers run at a
    # fraction of line rate)
    s1_d = nc.dram_tensor("s1", [A, PF + 2 * FA], bf16,
                          kind="ExternalInput")       # et | atoms b0 | b1
    s2_d = nc.dram_tensor("s2", [A, (NB - 4) * FA], bf16,
                          kind="ExternalInput")       # atoms b4..
    c1_d = nc.dram_tensor("c1", [A, 3 * C], bf16,
                          kind="ExternalInput")       # [W4;b4] 3 f-chunks
    c2_d = nc.dram_tensor("c2", [A, 2 * FA], bf16,
                          kind="ExternalInput")       # atoms b2 | b3
    bondm_d = nc.dram_tensor("bondm", [FB, D * MAIN], bf16,
                             kind="ExternalInput")
    aux_d = nc.dram_tensor("aux", [A, AUXC], bf16, kind="ExternalInput")
    wexc_d = nc.dram_tensor("wexc", [3, A, LC], bf16,
                            kind="ExternalInput")     # per f-chunk
    # main output, TRANSPOSED: out[r, ch*MAIN+s] = Z[slot s, C ch*128+r]
    out_d = nc.dram_tensor("out", [128, 2 * MAIN], bf16,
                           kind="ExternalOutput")
    oute_d = nc.dram_tensor("oute", [EXCU, C], bf16, kind="ExternalOutput")

    with tile.TileContext(nc) as tc, ExitStack() as ctx:
        consts = ctx.enter_context(tc.tile_pool(name="consts", bufs=1))
        pfeat = ctx.enter_context(tc.tile_pool(name="pfeat", bufs=1))
        pbs = ctx.enter_context(tc.tile_pool(name="pbs", bufs=2))
        pout = ctx.enter_context(tc.tile_pool(name="pout", bufs=3))
        ps_f = ctx.enter_context(
            tc.tile_pool(name="ps_f", bufs=1, space="PSUM"))
        ps_z = ctx.enter_context(
            tc.tile_pool(name="ps_z", bufs=2, space="PSUM"))
        ps_e = ctx.enter_context(
            tc.tile_pool(name="ps_e", bufs=1, space="PSUM"))

        t_s1 = consts.tile([A, PF + 2 * FA], bf16)
        t_s2 = consts.tile([A, (NB - 4) * FA], bf16)
        t_c1 = consts.tile([A, 3 * C], bf16)
        t_c2 = consts.tile([A, 2 * FA], bf16)
        bondm = pbs.tile([FB, D * MAIN], bf16)
        aux = consts.tile([A, AUXC], bf16)
        wexc = [consts.tile([A, LC], bf16, name=f"wexc{i}")
                for i in range(3)]

        # ---- input DMAs, need-ordered per queue: atoms first, then
        # weights/bonds, exception weights last ----------------------------
        nc.sync.dma_start(out=t_s1[:], in_=s1_d.ap()[:])
        nc.sync.dma_start(out=t_s2[:], in_=s2_d.ap()[:])
        nc.sync.dma_start(out=wexc[2][:], in_=wexc_d.ap()[2])
        nc.scalar.dma_start(out=t_c2[:], in_=c2_d.ap()[:])
        nc.scalar.dma_start(out=t_c1[:], in_=c1_d.ap()[:])
        nc.scalar.dma_start(out=wexc[1][:], in_=wexc_d.ap()[1])
        nc.gpsimd.dma_start(out=bondm[:], in_=bondm_d.ap()[:])
        nc.gpsimd.dma_start(out=aux[:], in_=aux_d.ap()[:])
        nc.gpsimd.dma_start(out=wexc[0][:], in_=wexc_d.ap()[0])

        def atile(b):
            if b < 2:
                return t_s1[:, PF + b * FA:PF + (b + 1) * FA]
            if b < 4:
                return t_c2[:, (b - 2) * FA:(b - 1) * FA]
            return t_s2[:, (b - 4) * FA:(b - 3) * FA]

        # ---- bond sums -> feature chunk2 ([bond rows; ones row]) ----------
        chunk2m = pfeat.tile([FAUG - 256, MAIN], bf16)
        nc.vector.memset(chunk2m[FB:FB + 1, :], 1.0)
        chunk2e = pfeat.tile([FAUG - 256, EXCU], bf16)
        nc.vector.memset(chunk2e[FB:FB + 1, :], 1.0)

        with nc.allow_low_precision(reason="bf16 bond sums, rel ~4e-3"):
            t01 = pbs.tile([FB, MAIN], bf16, tag="bt01")
            t23 = pbs.tile([FB, MAIN], bf16, tag="bt23")
            nc.vector.tensor_add(t01[:], bondm[:, 0:MAIN],
                                 bondm[:, MAIN:2 * MAIN])
            nc.vector.tensor_add(t23[:], bondm[:, 2 * MAIN:3 * MAIN],
                                 bondm[:, 3 * MAIN:4 * MAIN])
            nc.vector.tensor_add(t01[:], t01[:], t23[:])
            nc.vector.tensor_add(chunk2m[0:FB, :], t01[:],
                                 bondm[:, 4 * MAIN:5 * MAIN])
            e01 = pbs.tile([FB, EXCU], bf16, tag="be01")
            e23 = pbs.tile([FB, EXCU], bf16, tag="be23")
            nc.gpsimd.tensor_add(e01[:], aux[0:FB, 0:EXCU],
                                 aux[0:FB, EXCU:2 * EXCU])
            nc.gpsimd.tensor_add(e23[:], aux[0:FB, 2 * EXCU:3 * EXCU],
                                 aux[0:FB, 3 * EXCU:4 * EXCU])
            nc.gpsimd.tensor_add(e01[:], e01[:], e23[:])
            nc.gpsimd.tensor_add(chunk2e[0:FB, :], e01[:],
                                 aux[0:FB, 4 * EXCU:5 * EXCU])

        c2em = [pfeat.tile([FAUG - 256, EXCU], bf16, name=f"c2em{i}")
                for i in range(L)]
        with nc.allow_low_precision(reason="exact 0/1 masking"):
            for i in range(L):
                nc.gpsimd.tensor_mul(
                    c2em[i][:], chunk2e[:],
                    aux[0:FAUG - 256,
                        MDO + i * EXCU:MDO + (i + 1) * EXCU])

        # ---- neighbour+self gather: featT = atoms_bin.T @ ET --------------
        KB = SLOTB + EXB
        pf0 = ps_f.tile([A, PF], f32, tag="pf0")
        pf1 = ps_f.tile([A, PF], f32, tag="pf1")

        def gather(b):
            at = atile(b)
            ecols = t_s1[:, b * KB:(b + 1) * KB]
            nc.tensor.matmul(pf0[:, b * KB:(b + 1) * KB],
                             at[:, 0:128], ecols)
            nc.tensor.matmul(pf1[:, b * KB:(b + 1) * KB],
                             at[:, 128:256], ecols)

        # expected arrival order: b0,b1 (sync 1st), b2,b3 (scalar 1st),
        # b6.. (gpsimd 1st), b4,b5 (sync 2nd)
        gather(0)
        gather(1)
        gather(2)
        gather(3)
        for b in range(6, NB):
            gather(b)
        gather(4)
        gather(5)

        # compacted transposed features: main slots then exc slots
        featm0 = pfeat.tile([128, MAIN], bf16)
        featm1 = pfeat.tile([128, MAIN], bf16)
        fe0m = [pfeat.tile([128, EXCU], bf16, name=f"fe0m{i}")
                for i in range(L)]
        fe1m = [pfeat.tile([128, EXCU], bf16, name=f"fe1m{i}")
                for i in range(L)]
        with nc.allow_low_precision(reason="bf16 features, rel ~4e-3"):
            nc.scalar.copy(
                featm0.rearrange("p (j k) -> p j k", j=NB),
                pf0.rearrange("p (j k) -> p j k", j=NB)[:, :, 0:SLOTB])
            nc.vector.tensor_copy(
                featm1.rearrange("p (j k) -> p j k", j=NB),
                pf1.rearrange("p (j k) -> p j k", j=NB)[:, :, 0:SLOTB])
            for i in range(L):
                mcols = aux[:, MDO + i * EXCU:
                            MDO + (i + 1) * EXCU].rearrange(
                    "p (j k) -> p j k", j=EXBINS)
                nc.vector.tensor_mul(
                    fe0m[i].rearrange("p (j k) -> p j k", j=EXBINS),
                    pf0.rearrange("p (j k) -> p j k",
                                  j=NB)[:, 0:EXBINS, SLOTB:KB], mcols)
                nc.vector.tensor_mul(
                    fe1m[i].rearrange("p (j k) -> p j k", j=EXBINS),
                    pf1.rearrange("p (j k) -> p j k",
                                  j=NB)[:, 0:EXBINS, SLOTB:KB], mcols)

        # ---- main dense, weights stationary (transposed output):
        #      ZT[ch] = relu(W4[:, ch].T @ feat)  [128 C-rows, MAIN slots]
        outt = pout.tile([128, 2 * MAIN], bf16)
        for ch in range(2):
            co = ch * 384
            pz = ps_z.tile([128, MAIN], f32, tag=f"pzT{ch}")
            nc.tensor.matmul(pz[:], t_c1[:, co:co + 128], featm0[:],
                             start=True, stop=False)
            nc.tensor.matmul(pz[:], t_c1[:, co + 128:co + 256], featm1[:],
                             start=False, stop=False)
            nc.tensor.matmul(pz[:], t_c1[0:FAUG - 256, co + 256:co + 384],
                             chunk2m[:], start=False, stop=True)
            nc.scalar.activation(outt[:, ch * MAIN:(ch + 1) * MAIN], pz[:],
                                 AF.Relu)
        nc.sync.dma_start(out=out_d.ap()[:], in_=outt[:])

        # ---- exception slots: masked per-degree dense ---------------------
        if L > 0:
            # f-chunk-major so each group runs as its wexc chunk lands
            pze = ps_e.tile([EXCU, C], f32, tag="pze")
            for i in range(L):
                nc.tensor.matmul(pze[:], fe0m[i][:],
                                 wexc[0][:, i * C:(i + 1) * C],
                                 start=(i == 0), stop=False)
            for i in range(L):
                nc.tensor.matmul(pze[:], fe1m[i][:],
                                 wexc[1][:, i * C:(i + 1) * C],
                                 start=False, stop=False)
            for i in range(L):
                nc.tensor.matmul(pze[:], c2em[i][:],
                                 wexc[2][0:FAUG - 256, i * C:(i + 1) * C],
                                 start=False, stop=(i == L - 1))
            oute = pout.tile([EXCU, C], bf16)
            nc.scalar.activation(oute[:], pze[:], AF.Relu)
            nc.scalar.dma_start(out=oute_d.ap()[:], in_=oute[:])

    nc.compile()
    return nc


def _sparse_metadata(edges):
    """Host-side index metadata: bin packing + gather/sum matrices.

    Returns None if any capacity is exceeded (caller falls back to the
    dense program).
    """
    import ml_dtypes

    bf = ml_dtypes.bfloat16
    deg = (edges != -1).sum(axis=2)                      # (B, A)
    main_mask = deg == D - 1                             # deg == 4
    exc_mask = deg < D - 1                               # deg <= 3
    KB = SLOTB + EXB

    # per-core first-fit-decreasing packing; exception-bearing molecules
    # first-fit into bins 0..EXBINS-1 only
    per_core = []
    nb_max = 0
    for c in range(NCORES):
        stats = []
        for m in range(BL):
            bm = c * BL + m
            sel = np.nonzero(main_mask[bm])[0]
            exc = np.nonzero(exc_mask[bm])[0]
            if len(sel) + len(exc) == 0:
                continue
            uniq = set()
            for a in list(sel) + list(exc):
                uniq.add(int(a))
                for e in edges[bm, a]:
                    if e >= 0:
                        uniq.add(int(e))
            if len(uniq) > A or len(sel) > SLOTB or len(exc) > EXB:
                return None
            stats.append((m, len(uniq), len(sel), len(exc)))
        bins = []          # list of [rows, slots, excs, mols]
        ok = [True]

        def fit(m, nr, nsl, nex, hi):
            for bi in range(min(hi, len(bins))):
                r, s, e, mols = bins[bi]
                if r + nr <= A and s + nsl <= SLOTB and e + nex <= EXB:
                    bins[bi] = [r + nr, s + nsl, e + nex, mols + [m]]
                    return
            if len(bins) < hi:
                bins.append([nr, nsl, nex, [m]])
                return
            ok[0] = False

        for m, nr, nsl, nex in [t for t in stats if t[3] > 0]:
            fit(m, nr, nsl, nex, EXBINS)
        for m, nr, nsl, nex in sorted([t for t in stats if t[3] == 0],
                                      key=lambda t: -t[1]):
            fit(m, nr, nsl, nex, NBMAX)
        if not ok[0]:
            return None
        per_core.append([b[3] for b in bins])
        nb_max = max(nb_max, len(bins))
    NB = nb_max
    MAIN = NB * SLOTB

    et = np.zeros((NCORES, A, NB * KB), dtype=np.float32)
    gidx = np.zeros((NCORES, NB, A), dtype=np.int64)
    main_rows = [[] for _ in range(NCORES)]   # (slot, mol, atom)
    exc_rows = [[] for _ in range(NCORES)]    # (eslot, mol, atom)
    bidx_m = np.zeros((NCORES, MAIN), dtype=np.int64)
    bval_m = np.zeros((NCORES, MAIN), dtype=bool)
    bidx_e = np.zeros((NCORES, EXCU), dtype=np.int64)
    bval_e = np.zeros((NCORES, EXCU), dtype=bool)
    excdeg = {}

    for c in range(NCORES):
        for bnum, mols in enumerate(per_core[c]):
            off = 0            # bin-local gathered-atom row
            k = 0              # main slot within bin
            ke = 0             # exc slot within bin
            loc = {}
            uniq = []
            for m in mols:
                bm = c * BL + m
                sel = np.nonzero(main_mask[bm])[0]
                exc = np.nonzero(exc_mask[bm])[0]
                for a in list(sel) + list(exc):
                    srcs = [int(a)] + [int(e) for e in edges[bm, a] if e >= 0]
                    for s2 in srcs:
                        key = (m, s2)
                        if key not in loc:
                            loc[key] = len(uniq)
                            uniq.append(key)
                    if deg[bm, a] == D - 1:
                        col = bnum * KB + k
                        slot = bnum * SLOTB + k
                        k += 1
                        main_rows[c].append((slot, m, int(a)))
                        bidx_m[c, slot] = m * A + int(a)
                        bval_m[c, slot] = True
                    else:
                        col = bnum * KB + SLOTB + ke
                        eslot = bnum * EXB + ke
                        ke += 1
                        exc_rows[c].append((eslot, m, int(a)))
                        bidx_e[c, eslot] = m * A + int(a)
                        bval_e[c, eslot] = True
                        excdeg[(c, eslot)] = int(deg[bm, a])
                    for s2 in srcs:
                        et[c, loc[(m, s2)], col] += 1.0
            assert len(uniq) <= A
            for r, (m, s2) in enumerate(uniq):
                gidx[c, bnum, r] = m * A + s2

    dlist = sorted(set(excdeg.values()))
    L = len(dlist)
    LW = max(L, 1)
    md = np.zeros((NCORES, A, LW * EXCU), dtype=np.float32)
    for (c, eslot), d in excdeg.items():
        md[c, :, dlist.index(d) * EXCU + eslot] = 1.0
    return {
        "NB": NB,
        "dlist": dlist,
        "et": et.astype(bf),
        "md": md.astype(bf),
        "gidx": gidx,
        "main_rows": main_rows,
        "exc_rows": exc_rows,
        "bidx_m": bidx_m,
        "bval_m": bval_m,
        "bidx_e": bidx_e,
        "bval_e": bval_e,
    }


def _make_in_maps_sparse(atoms, bonds, W, b, meta):
    import ml_dtypes

    bf = ml_dtypes.bfloat16
    NB = meta["NB"]
    MAIN = NB * SLOTB
    PF = NB * (SLOTB + EXB)
    L = len(meta["dlist"])
    LW = max(L, 1)
    LC = LW * C
    AUXC = D * EXCU + LW * EXCU

    atoms_flat = atoms.reshape(NCORES, BL * A, FA)
    # gathered atoms, partition-major: [NCORES, A, NB, FA]
    atoms8 = atoms_flat[np.arange(NCORES)[:, None, None],
                        meta["gidx"]].transpose(0, 2, 1, 3)

    waug = np.concatenate([W, b[:, None, :]], axis=1)     # (5, 321, 256)
    # ch-major: [ch half][f chunk] of 128 cols each
    wd4p = np.zeros((128, 3 * C), dtype=np.float32)
    for ch in range(2):
        cs = ch * 128
        wd4p[:, ch * 384:ch * 384 + 128] = waug[D - 1][0:128, cs:cs + 128]
        wd4p[:, ch * 384 + 128:ch * 384 + 256] = \
            waug[D - 1][128:256, cs:cs + 128]
        wd4p[0:FAUG - 256, ch * 384 + 256:ch * 384 + 384] = \
            waug[D - 1][256:FAUG, cs:cs + 128]
    wexc = np.zeros((3, 128, LC), dtype=np.float32)       # per f-chunk
    for i, d in enumerate(meta["dlist"]):
        wexc[0, :, i * C:(i + 1) * C] = waug[d][0:128]
        wexc[1, :, i * C:(i + 1) * C] = waug[d][128:256]
        wexc[2, 0:FAUG - 256, i * C:(i + 1) * C] = waug[d][256:FAUG]
    wd4p = wd4p.astype(bf)
    wexc = wexc.astype(bf)

    bonds_flat = bonds.reshape(NCORES, BL * A, D, FB)
    in_maps = []
    for c in range(NCORES):
        bm = bonds_flat[c][meta["bidx_m"][c]]             # (MAIN, D, FB)
        bm = bm * meta["bval_m"][c][:, None, None]
        be = bonds_flat[c][meta["bidx_e"][c]]             # (EXCU, D, FB)
        be = be * meta["bval_e"][c][:, None, None]
        bondm = np.ascontiguousarray(
            bm.transpose(2, 1, 0).reshape(FB, D * MAIN)).astype(bf)
        auxp = np.zeros((A, AUXC), dtype=np.float32)
        auxp[0:FB, 0:D * EXCU] = be.transpose(2, 1, 0).reshape(FB, D * EXCU)
        auxp[:, D * EXCU:AUXC] = meta["md"][c]

        s1 = np.zeros((A, PF + 2 * FA), dtype=np.float32)
        s1[:, 0:PF] = meta["et"][c]
        s1[:, PF:PF + FA] = atoms8[c, :, 0]
        s1[:, PF + FA:PF + 2 * FA] = atoms8[c, :, 1]
        s2 = atoms8[c, :, 4:NB].reshape(A, (NB - 4) * FA)
        c2 = np.concatenate([atoms8[c, :, 2], atoms8[c, :, 3]], axis=1)
        im = {
            "s1": s1.astype(bf),
            "s2": np.ascontiguousarray(s2).astype(bf),  # bins 4..
            "c1": wd4p,
            "c2": np.ascontiguousarray(c2).astype(bf),
            "bondm": bondm,
            "aux": auxp.astype(bf),
            "wexc": wexc,
        }
        in_maps.append(im)
    return in_maps


# ---------------------------------------------------------------------------
# dense fallback program (unchanged from the baseline kernel)
# ---------------------------------------------------------------------------

def _build_program_dense():
    from contextlib import ExitStack

    import concourse.bass as bass
    import concourse.tile as tile
    from concourse import bacc, mybir

    f32 = mybir.dt.float32
    AF = mybir.ActivationFunctionType
    OP = mybir.AluOpType
    f32r = mybir.dt.float32r
    bf16 = mybir.dt.bfloat16

    nc = bacc.Bacc("TRN2", target_bir_lowering=False, debug=False,
                   num_devices=NCORES)

    atoms_d = nc.dram_tensor("atoms", [BL, A, FA], f32r, kind="ExternalInput")
    bonds_d = nc.dram_tensor("bonds", [BL, A, D * FB], f32,
                             kind="ExternalInput")
    edges_d = nc.dram_tensor("edges", [BL, A, A * D], bf16,
                             kind="ExternalInput")
    waug_d = nc.dram_tensor("waug", [D, FAUG, C], f32r, kind="ExternalInput")
    ident_d = nc.dram_tensor("ident", [A, A], f32, kind="ExternalInput")
    identr_d = nc.dram_tensor("identr", [A, A], f32r, kind="ExternalInput")
    identb_d = nc.dram_tensor("identb", [A, A], bf16, kind="ExternalInput")
    iota_d = nc.dram_tensor("iota", [A, 1], f32, kind="ExternalInput")
    edeg_d = nc.dram_tensor("edeg", [BL, A, D], f32, kind="ExternalInput")
    onesr_d = nc.dram_tensor("onesr", [1, A], f32, kind="ExternalInput")
    out_d = nc.dram_tensor("out", [BL, A, C], f32, kind="ExternalOutput")

    atoms_ap = atoms_d.ap()
    bonds_ap = bonds_d.ap()
    edges_ap = edges_d.ap()
    out_ap = out_d.ap()

    with tile.TileContext(nc) as tc, ExitStack() as ctx:
        consts = ctx.enter_context(tc.tile_pool(name="consts", bufs=1))
        pin = ctx.enter_context(tc.tile_pool(name="pin", bufs=3))
        pbc = ctx.enter_context(tc.tile_pool(name="pbc", bufs=2))
        pet = ctx.enter_context(tc.tile_pool(name="pet", bufs=2))
        pfeat = ctx.enter_context(tc.tile_pool(name="pfeat", bufs=2))
        pmd = ctx.enter_context(tc.tile_pool(name="pmd", bufs=2))
        pz = ctx.enter_context(tc.tile_pool(name="pz", bufs=2))
        pout = ctx.enter_context(tc.tile_pool(name="pout", bufs=3))
        ps_f = ctx.enter_context(
            tc.tile_pool(name="ps_f", bufs=2, space="PSUM"))
        ps_c2 = ctx.enter_context(
            tc.tile_pool(name="ps_c2", bufs=1, space="PSUM"))
        ps_z = ctx.enter_context(
            tc.tile_pool(name="ps_z", bufs=1, space="PSUM"))
        ps_s = ctx.enter_context(
            tc.tile_pool(name="ps_s", bufs=1, space="PSUM"))

        G4 = 4
        ident = consts.tile([A, A], f32)
        nc.scalar.dma_start(out=ident[:], in_=ident_d.ap()[:])
        identr = consts.tile([A, A], f32r)
        nc.scalar.dma_start(out=identr[:], in_=identr_d.ap()[:])
        iota_col = consts.tile([A, 1], f32)
        nc.gpsimd.dma_start(out=iota_col[:], in_=iota_d.ap()[:])
        ones_row = consts.tile([1, A], f32)
        nc.scalar.dma_start(out=ones_row[:], in_=onesr_d.ap()[:])
        identb4 = consts.tile([A, G4 * A], bf16)
        for j in range(G4):
            nc.gpsimd.dma_start(out=identb4[:, j * A:(j + 1) * A],
                                in_=identb_d.ap()[:])

        w0 = consts.tile([128, D * C], f32r)
        w1 = consts.tile([128, D * C], f32r)
        w2 = consts.tile([FAUG - 256, D * C], f32r)
        for d in range(D):
            nc.scalar.dma_start(out=w0[:, d * C:(d + 1) * C],
                                in_=waug_d.ap()[d, 0:128, :])
            nc.scalar.dma_start(out=w1[:, d * C:(d + 1) * C],
                                in_=waug_d.ap()[d, 128:256, :])
            nc.scalar.dma_start(out=w2[:, d * C:(d + 1) * C],
                                in_=waug_d.ap()[d, 256:FAUG, :])

        for bg in range(BL // G4):
            mols = range(bg * G4, (bg + 1) * G4)
            atoms4 = pin.tile([A, G4 * FA], f32r)
            nc.sync.dma_start(
                out=atoms4.rearrange("p (g f) -> p g f", g=G4),
                in_=atoms_ap[bg * G4:(bg + 1) * G4].rearrange(
                    "g p f -> p g f"))
            bonds4 = pin.tile([A, G4 * D * FB], f32)
            nc.sync.dma_start(
                out=bonds4.rearrange("p (g f) -> p g f", g=G4),
                in_=bonds_ap[bg * G4:(bg + 1) * G4].rearrange(
                    "g p f -> p g f"))
            bc_e4 = pbc.tile([A, G4 * A * D], bf16)
            nc.gpsimd.dma_start(
                out=bc_e4.rearrange("p (g f) -> p g f", g=G4),
                in_=edges_ap[bg * G4:(bg + 1) * G4].rearrange(
                    "g p f -> p g f"))
            edeg4 = pfeat.tile([A, G4 * D], f32)
            nc.sync.dma_start(
                out=edeg4.rearrange("p (g f) -> p g f", g=G4),
                in_=edeg_d.ap()[bg * G4:(bg + 1) * G4].rearrange(
                    "g p f -> p g f"))
            ne4 = pfeat.tile([A, G4 * D], f32)
            nc.vector.tensor_scalar(ne4[:], edeg4[:], -1.0, None,
                                    OP.not_equal)
            degp1_4 = pfeat.tile([A, G4], f32)
            nc.vector.tensor_reduce(
                degp1_4[:], ne4.rearrange("p (g d) -> p g d", g=G4),
                axis=mybir.AxisListType.X, op=OP.add)
            nc.vector.tensor_scalar(degp1_4[:], degp1_4[:], 1.0, None,
                                    OP.add)

            cmp5 = pbc.tile([A, G4 * A * D], bf16)
            nc.vector.tensor_scalar(cmp5[:], bc_e4[:], iota_col[:], None,
                                    OP.is_equal)
            cg = cmp5.rearrange("p (g d a) -> p g d a", g=G4, d=D)
            t01 = pet.tile([A, G4 * A], bf16)
            nc.vector.tensor_add(t01[:], cg[:, :, 0, :], cg[:, :, 1, :])
            t23 = pet.tile([A, G4 * A], bf16)
            nc.vector.tensor_add(t23[:], cg[:, :, 2, :], cg[:, :, 3, :])
            t4i = pet.tile([A, G4 * A], bf16)
            nc.vector.tensor_add(t4i[:], cg[:, :, 4, :], identb4[:])
            t0123 = pet.tile([A, G4 * A], bf16)
            nc.vector.tensor_add(t0123[:], t01[:], t23[:])
            etp4 = pet.tile([A, G4 * A], f32r)
            with nc.allow_low_precision(reason="exact small-int counts"):
                nc.vector.tensor_add(etp4[:], t0123[:], t4i[:])

            out4 = pout.tile([A, G4 * C], f32)
            for j, bm in enumerate(mols):
                etp = etp4[:, j * A:(j + 1) * A]
                atoms_sb = atoms4[:, j * FA:(j + 1) * FA]
                bonds_sb = bonds4[:, j * D * FB:(j + 1) * D * FB]

                degp1 = degp1_4[:, j:j + 1]

                pf = ps_f.tile([A, FA], f32)
                nc.tensor.matmul(pf[:, 0:128], atoms_sb[:, 0:128], etp)
                nc.tensor.matmul(pf[:, 128:256], atoms_sb[:, 128:256], etp)

                featT01 = pfeat.tile([A, FA], f32r)
                nc.scalar.copy(featT01[:], pf[:, 0:FA])

                sumbond = pfeat.tile([A, FB], f32r)
                with nc.allow_low_precision(
                        reason="f32r rounding of bond sums"):
                    nc.vector.reduce_sum(
                        sumbond[:],
                        bonds_sb.rearrange("p (d f) -> p f d", d=D),
                        axis=mybir.AxisListType.X)
                pc2 = ps_c2.tile([FB, A], f32)
                nc.tensor.matmul(pc2[:], sumbond[:], identr[:])
                chunk2 = pfeat.tile([FAUG - 256, A], f32r)
                nc.scalar.copy(chunk2[0:FB, :], pc2[:])
                nc.vector.tensor_copy(chunk2[FB:FB + 1, :], ones_row[:])

                md = pmd.tile([A, D * A], f32r)
                for d in range(D):
                    nc.vector.tensor_scalar(md[:, d * A:(d + 1) * A],
                                            ident[:], degp1[:], float(d + 1),
                                            OP.mult, OP.is_equal)

                lhs = [featT01[:, 0:128], featT01[:, 128:256], chunk2[:]]
                rhs = [w0, w1, w2]
                groups = [(0, 512), (512, 1024), (1024, 1280)]
                zsb = pz.tile([A, D * C], f32r)
                for g0, g1 in groups:
                    pzg = ps_z.tile([A, 512], f32, tag="pzg", bufs=4)
                    nc.tensor.matmul(pzg[:, 0:g1 - g0], lhs[0],
                                     rhs[0][:, g0:g1], start=True, stop=False)
                    nc.tensor.matmul(pzg[:, 0:g1 - g0], lhs[1],
                                     rhs[1][:, g0:g1], start=False,
                                     stop=False)
                    nc.tensor.matmul(pzg[:, 0:g1 - g0], lhs[2],
                                     rhs[2][:, g0:g1], start=False, stop=True)
                    nc.scalar.copy(zsb[:, g0:g1], pzg[:, 0:g1 - g0])

                pst = ps_s.tile([A, C], f32)
                for d in range(D):
                    nc.tensor.matmul(pst[:], md[:, d * A:(d + 1) * A],
                                     zsb[:, d * C:(d + 1) * C],
                                     start=(d == 0), stop=(d == D - 1))
                nc.scalar.activation(out4[:, j * C:(j + 1) * C], pst[:],
                                     AF.Relu)
            nc.gpsimd.dma_start(
                out=out_ap[bg * G4:(bg + 1) * G4].rearrange("g p f -> p g f"),
                in_=out4.rearrange("p (g f) -> p g f", g=G4))

    nc.compile()
    return nc


def _make_in_maps_dense(atoms, bonds, edges, W, b):
    atoms = np.ascontiguousarray(np.asarray(atoms, dtype=np.float32))
    bonds = np.ascontiguousarray(np.asarray(bonds, dtype=np.float32))
    edges = np.asarray(edges)
    W = np.asarray(W, dtype=np.float32)
    b = np.asarray(b, dtype=np.float32)

    import ml_dtypes
    edges_f = np.ascontiguousarray(edges.transpose(0, 2, 1)).reshape(
        B, D * A).astype(ml_dtypes.bfloat16)
    edges_rep = np.ascontiguousarray(
        np.broadcast_to(edges_f[:, None, :], (B, A, D * A)))

    waug = np.ascontiguousarray(
        np.concatenate([W, b[:, None, :]], axis=1))           # (5, 321, 256)
    ident = np.eye(A, dtype=np.float32)
    iota = np.arange(A, dtype=np.float32).reshape(A, 1)
    onesr = np.ones((1, A), dtype=np.float32)

    edeg8 = edges.reshape(NCORES, BL, A, D).astype(np.float32)
    atoms8 = atoms.reshape(NCORES, BL, A, FA)
    bonds8 = bonds.reshape(NCORES, BL, A, D * FB)
    edges8 = edges_rep.reshape(NCORES, BL, A, A * D)

    return [
        {
            "atoms": atoms8[c],
            "bonds": bonds8[c],
            "edges": edges8[c],
            "waug": waug,
            "ident": ident,
            "identr": ident,
            "identb": ident.astype(ml_dtypes.bfloat16),
            "iota": iota,
            "edeg": edeg8[c],
            "onesr": onesr,
        }
        for c in range(NCORES)
    ]


# ---------------------------------------------------------------------------
# entry points
# ---------------------------------------------------------------------------

def run_sharded(atoms, bonds, edges, W, b, trace=False):
    """Run on the 8 NeuronCores; returns (output, BassKernelResults)."""
    from concourse.bass_utils import run_bass_kernel_spmd

    atoms = np.ascontiguousarray(np.asarray(atoms, dtype=np.float32))
    bonds = np.ascontiguousarray(np.asarray(bonds, dtype=np.float32))
    edges = np.asarray(edges)
    W = np.asarray(W, dtype=np.float32)
    b = np.asarray(b, dtype=np.float32)

    meta = _sparse_metadata(edges)
    if meta is None:
        if "dense" not in _CACHE:
            _CACHE["dense"] = _build_program_dense()
        nc = _CACHE["dense"]
        in_maps = _make_in_maps_dense(atoms, bonds, edges, W, b)
        res = run_bass_kernel_spmd(nc, in_maps, list(range(NCORES)),
                                   trace=trace)
        out = np.concatenate(
            [res.results[c]["out"] for c in range(NCORES)],
            axis=0).reshape(B, A, C)
        return out, res

    L = len(meta["dlist"])
    NB = meta["NB"]
    key = ("sparse", L, NB)
    if key not in _CACHE:
        _CACHE[key] = _build_program_sparse(L, NB)
    nc = _CACHE[key]
    in_maps = _make_in_maps_sparse(atoms, bonds, W, b, meta)
    res = run_bass_kernel_spmd(nc, in_maps, list(range(NCORES)), trace=trace)

    MAIN = NB * SLOTB
    out = np.zeros((B, A, C), dtype=np.float32)
    for c in range(NCORES):
        # out is transposed on device: [128, 2*MAIN], C row = ch*128+r
        arr = np.asarray(res.results[c]["out"], dtype=np.float32)
        zt = np.concatenate([arr[:, 0:MAIN], arr[:, MAIN:2 * MAIN]], axis=0)
        main = zt.T                                      # [MAIN, C]
        exc = np.asarray(res.results[c]["oute"], dtype=np.float32)
        for slot, ml, a in meta["main_rows"][c]:
            out[c * BL + ml, a] = main[slot]
        for eslot, ml, a in meta["exc_rows"][c]:
            out[c * BL + ml, a] = exc[eslot]
    return out, res


def kernel(atoms, bonds, edges, W, b):
    out, _ = run_sharded(atoms, bonds, edges, W, b)
    return out


# revision 38
# speedup vs baseline: 1.2006x; 1.0553x over previous
"""Trainium2 Bass kernel for NeuralGraphHidden (GNN message passing).

Full-input contract: kernel(**inputs) takes the complete unsharded arrays,
shards batch dim 0 across 8 NeuronCores (data parallel), runs one SPMD Bass
program, and reassembles the full output.

Math (per molecule, A=128 atoms, D=5 degree slots):
  deg[a]   = #(edges[a,:] != -1)
  out[a]   = relu(feat[a] @ W[deg[a]] + b[deg[a]])  if deg[a] < 5 else 0
  feat[a]  = [atoms[a] + sum_d atoms[edges[a,d]],  sum_d bonds[a,d]]

Degree-sparsity: deg[a] == 5 makes every mask (deg == d), d < 5, zero ->
the output row is structurally zero.  ~96% of atoms.  Of the rest, almost
all have deg == 4, so the kernel splits:
  main slots: deg == 4 atoms -> one dense with W[4]
  exc  slots: deg  < 4 atoms (a handful per core) -> masked multi-degree
              dense with the present degrees' weights

host (index metadata only, no float math): per core, molecules are
permuted so exception-bearing ones come first, then greedily packed into
bins of <=128 unique referenced atoms, <=24 main slots, <=4 exc slots
(exc only in the first 3 bins).  Gather/sum matrices ET[src_local, slot]
hold small integer counts.  Bond rows are gathered per slot.  All float
values flow through the device (bf16 operands / f32 accumulate); outputs
come back bf16 and are scattered into a zero f32 array.

device: featT = atoms_bin.T @ ET (TensorE), bond sums (VectorE),
Z = relu(featT.T @ [W4;b4]) for two 128-slot groups, masked per-degree
accumulated dense for exc slots.  DMAs are need-ordered across the three
queues (weights first); idle warm-up matmuls keep the PE HAM clock up.

A dense all-degrees fallback program is kept for the (never observed)
case that a capacity is exceeded.
"""

import sys

sys.path.insert(0, "/opt/trn_rl_repo")

import numpy as np

B, A, D = 256, 128, 5
FA, FB, C = 256, 64, 256
F = FA + FB        # 320
FAUG = F + 1       # 321 (bias row)
NCORES = 8
BL = B // NCORES   # 32 molecules per core

SLOTB = 26         # main slots per bin (deg==4 atoms)
EXB = 4            # exc slots per bin (deg<4 atoms)
EXBINS = 3         # only the first EXBINS bins may hold exc slots
EXCU = EXBINS * EXB  # 12 exc slots per core
NBMAX = 10         # bin-count cap (program is compiled per actual NB)

_CACHE = {}


# ---------------------------------------------------------------------------
# sparse program
# ---------------------------------------------------------------------------

def _build_program_sparse(L, NB):
    """L = #distinct exception degrees; NB = atom bins per core."""
    from contextlib import ExitStack

    import concourse.bass as bass
    import concourse.tile as tile
    from concourse import bacc, mybir

    f32 = mybir.dt.float32
    AF = mybir.ActivationFunctionType
    bf16 = mybir.dt.bfloat16

    MAIN = NB * SLOTB          # main slot columns (<=260)
    PF = NB * (SLOTB + EXB)    # psum gather columns per feature chunk
    LW = max(L, 1)
    LC = LW * C
    AUXC = D * EXCU + LW * EXCU
    MDO = D * EXCU             # aux col offset of the md masks
    assert 6 <= NB <= NBMAX

    nc = bacc.Bacc("TRN2", target_bir_lowering=False, debug=False,
                   num_devices=NCORES)

    # medium-size per-DMA tensors, each with per-partition-contiguous
    # layout (strided or sub-512B-per-partition transfers run at a
    # fraction of line rate)
    s1_d = nc.dram_tensor("s1", [A, PF + 2 * FA], bf16,
                          kind="ExternalInput")       # et | atoms b0 | b1
    s2_d = nc.dram_tensor("s2", [A, (NB - 4) * FA], bf16,
                          kind="ExternalInput")       # atoms b4..
    c1_d = nc.dram_tensor("c1", [A, 3 * C], bf16,
                          kind="ExternalInput")       # [W4;b4] 3 f-chunks
    c2_d = nc.dram_tensor("c2", [A, 2 * FA], bf16,
                          kind="ExternalInput")       # atoms b2 | b3
    bondm_d = nc.dram_tensor("bondm", [FB, D * MAIN], bf16,
                             kind="ExternalInput")
    aux_d = nc.dram_tensor("aux", [A, AUXC], bf16, kind="ExternalInput")
    wexc_d = nc.dram_tensor("wexc", [3, A, LC], bf16,
                            kind="ExternalInput")     # per f-chunk
    # main output, TRANSPOSED: out[ch, r, s] = Z[slot s, C ch*128+r]
    out_d = nc.dram_tensor("out", [2, 128, MAIN], bf16,
                           kind="ExternalOutput")
    oute_d = nc.dram_tensor("oute", [EXCU, C], bf16, kind="ExternalOutput")

    with tile.TileContext(nc) as tc, ExitStack() as ctx:
        consts = ctx.enter_context(tc.tile_pool(name="consts", bufs=1))
        pfeat = ctx.enter_context(tc.tile_pool(name="pfeat", bufs=1))
        pbs = ctx.enter_context(tc.tile_pool(name="pbs", bufs=2))
        pout = ctx.enter_context(tc.tile_pool(name="pout", bufs=3))
        ps_f = ctx.enter_context(
            tc.tile_pool(name="ps_f", bufs=1, space="PSUM"))
        ps_z = ctx.enter_context(
            tc.tile_pool(name="ps_z", bufs=2, space="PSUM"))
        ps_e = ctx.enter_context(
            tc.tile_pool(name="ps_e", bufs=1, space="PSUM"))

        t_s1 = consts.tile([A, PF + 2 * FA], bf16)
        t_s2 = consts.tile([A, (NB - 4) * FA], bf16)
        t_c1 = consts.tile([A, 3 * C], bf16)
        t_c2 = consts.tile([A, 2 * FA], bf16)
        bondm = pbs.tile([FB, D * MAIN], bf16)
        aux = consts.tile([A, AUXC], bf16)
        wexc = [consts.tile([A, LC], bf16, name=f"wexc{i}")
                for i in range(3)]

        # ---- input DMAs, need-ordered per queue: atoms first, then
        # weights/bonds, exception weights last ----------------------------
        nc.sync.dma_start(out=t_s1[:], in_=s1_d.ap()[:])
        nc.sync.dma_start(out=t_s2[:], in_=s2_d.ap()[:])
        nc.sync.dma_start(out=wexc[2][:], in_=wexc_d.ap()[2])
        nc.scalar.dma_start(out=t_c2[:], in_=c2_d.ap()[:])
        nc.scalar.dma_start(out=t_c1[:], in_=c1_d.ap()[:])
        nc.scalar.dma_start(out=wexc[1][:], in_=wexc_d.ap()[1])
        nc.gpsimd.dma_start(out=bondm[:], in_=bondm_d.ap()[:])
        nc.gpsimd.dma_start(out=aux[:], in_=aux_d.ap()[:])
        nc.gpsimd.dma_start(out=wexc[0][:], in_=wexc_d.ap()[0])

        def atile(b):
            if b < 2:
                return t_s1[:, PF + b * FA:PF + (b + 1) * FA]
            if b < 4:
                return t_c2[:, (b - 2) * FA:(b - 1) * FA]
            return t_s2[:, (b - 4) * FA:(b - 3) * FA]

        # ---- bond sums -> feature chunk2 ([bond rows; ones row]) ----------
        chunk2m = pfeat.tile([FAUG - 256, MAIN], bf16)
        nc.vector.memset(chunk2m[FB:FB + 1, :], 1.0)
        chunk2e = pfeat.tile([FAUG - 256, EXCU], bf16)
        nc.vector.memset(chunk2e[FB:FB + 1, :], 1.0)

        with nc.allow_low_precision(reason="bf16 bond sums, rel ~4e-3"):
            t01 = pbs.tile([FB, MAIN], bf16, tag="bt01")
            t23 = pbs.tile([FB, MAIN], bf16, tag="bt23")
            nc.vector.tensor_add(t01[:], bondm[:, 0:MAIN],
                                 bondm[:, MAIN:2 * MAIN])
            nc.vector.tensor_add(t23[:], bondm[:, 2 * MAIN:3 * MAIN],
                                 bondm[:, 3 * MAIN:4 * MAIN])
            nc.vector.tensor_add(t01[:], t01[:], t23[:])
            nc.vector.tensor_add(chunk2m[0:FB, :], t01[:],
                                 bondm[:, 4 * MAIN:5 * MAIN])
            e01 = pbs.tile([FB, EXCU], bf16, tag="be01")
            e23 = pbs.tile([FB, EXCU], bf16, tag="be23")
            nc.gpsimd.tensor_add(e01[:], aux[0:FB, 0:EXCU],
                                 aux[0:FB, EXCU:2 * EXCU])
            nc.gpsimd.tensor_add(e23[:], aux[0:FB, 2 * EXCU:3 * EXCU],
                                 aux[0:FB, 3 * EXCU:4 * EXCU])
            nc.gpsimd.tensor_add(e01[:], e01[:], e23[:])
            nc.gpsimd.tensor_add(chunk2e[0:FB, :], e01[:],
                                 aux[0:FB, 4 * EXCU:5 * EXCU])

        c2em = [pfeat.tile([FAUG - 256, EXCU], bf16, name=f"c2em{i}")
                for i in range(L)]
        with nc.allow_low_precision(reason="exact 0/1 masking"):
            for i in range(L):
                nc.gpsimd.tensor_mul(
                    c2em[i][:], chunk2e[:],
                    aux[0:FAUG - 256,
                        MDO + i * EXCU:MDO + (i + 1) * EXCU])

        # ---- neighbour+self gather: featT = atoms_bin.T @ ET --------------
        KB = SLOTB + EXB
        pf0 = ps_f.tile([A, PF], f32, tag="pf0")
        pf1 = ps_f.tile([A, PF], f32, tag="pf1")

        def gather(b):
            at = atile(b)
            ecols = t_s1[:, b * KB:(b + 1) * KB]
            nc.tensor.matmul(pf0[:, b * KB:(b + 1) * KB],
                             at[:, 0:128], ecols)
            nc.tensor.matmul(pf1[:, b * KB:(b + 1) * KB],
                             at[:, 128:256], ecols)

        # expected arrival order: b0,b1 (sync 1st), b2,b3 (scalar 1st),
        # b6.. (gpsimd 1st), b4,b5 (sync 2nd)
        gather(0)
        gather(1)
        gather(2)
        gather(3)
        for b in range(6, NB):
            gather(b)
        gather(4)
        gather(5)

        # compacted transposed features: main slots then exc slots
        featm0 = pfeat.tile([128, MAIN], bf16)
        featm1 = pfeat.tile([128, MAIN], bf16)
        fe0m = [pfeat.tile([128, EXCU], bf16, name=f"fe0m{i}")
                for i in range(L)]
        fe1m = [pfeat.tile([128, EXCU], bf16, name=f"fe1m{i}")
                for i in range(L)]
        with nc.allow_low_precision(reason="bf16 features, rel ~4e-3"):
            nc.scalar.copy(
                featm0.rearrange("p (j k) -> p j k", j=NB),
                pf0.rearrange("p (j k) -> p j k", j=NB)[:, :, 0:SLOTB])
            nc.vector.tensor_copy(
                featm1.rearrange("p (j k) -> p j k", j=NB),
                pf1.rearrange("p (j k) -> p j k", j=NB)[:, :, 0:SLOTB])
            for i in range(L):
                mcols = aux[:, MDO + i * EXCU:
                            MDO + (i + 1) * EXCU].rearrange(
                    "p (j k) -> p j k", j=EXBINS)
                nc.vector.tensor_mul(
                    fe0m[i].rearrange("p (j k) -> p j k", j=EXBINS),
                    pf0.rearrange("p (j k) -> p j k",
                                  j=NB)[:, 0:EXBINS, SLOTB:KB], mcols)
                nc.vector.tensor_mul(
                    fe1m[i].rearrange("p (j k) -> p j k", j=EXBINS),
                    pf1.rearrange("p (j k) -> p j k",
                                  j=NB)[:, 0:EXBINS, SLOTB:KB], mcols)

        # ---- main dense, weights stationary (transposed output):
        #      ZT[ch] = relu(W4[:, ch].T @ feat)  [128 C-rows, MAIN slots]
        outt = pout.tile([128, 2 * MAIN], bf16)
        out_eng = [nc.sync, nc.scalar]
        for ch in range(2):
            co = ch * 384
            pz = ps_z.tile([128, MAIN], f32, tag=f"pzT{ch}")
            nc.tensor.matmul(pz[:], t_c1[:, co:co + 128], featm0[:],
                             start=True, stop=False)
            nc.tensor.matmul(pz[:], t_c1[:, co + 128:co + 256], featm1[:],
                             start=False, stop=False)
            nc.tensor.matmul(pz[:], t_c1[0:FAUG - 256, co + 256:co + 384],
                             chunk2m[:], start=False, stop=True)
            nc.scalar.activation(outt[:, ch * MAIN:(ch + 1) * MAIN], pz[:],
                                 AF.Relu)
            out_eng[ch].dma_start(out=out_d.ap()[ch],
                                  in_=outt[:, ch * MAIN:(ch + 1) * MAIN])

        # ---- exception slots: masked per-degree dense ---------------------
        if L > 0:
            # f-chunk-major so each group runs as its wexc chunk lands
            pze = ps_e.tile([EXCU, C], f32, tag="pze")
            for i in range(L):
                nc.tensor.matmul(pze[:], fe0m[i][:],
                                 wexc[0][:, i * C:(i + 1) * C],
                                 start=(i == 0), stop=False)
            for i in range(L):
                nc.tensor.matmul(pze[:], fe1m[i][:],
                                 wexc[1][:, i * C:(i + 1) * C],
                                 start=False, stop=False)
            for i in range(L):
                nc.tensor.matmul(pze[:], c2em[i][:],
                                 wexc[2][0:FAUG - 256, i * C:(i + 1) * C],
                                 start=False, stop=(i == L - 1))
            oute = pout.tile([EXCU, C], bf16)
            nc.scalar.activation(oute[:], pze[:], AF.Relu)
            nc.sync.dma_start(out=oute_d.ap()[:], in_=oute[:])

    nc.compile()
    return nc


def _sparse_metadata(edges):
    """Host-side index metadata: bin packing + gather/sum matrices.

    Returns None if any capacity is exceeded (caller falls back to the
    dense program).
    """
    import ml_dtypes

    bf = ml_dtypes.bfloat16
    deg = (edges != -1).sum(axis=2)                      # (B, A)
    main_mask = deg == D - 1                             # deg == 4
    exc_mask = deg < D - 1                               # deg <= 3
    KB = SLOTB + EXB

    # per-core first-fit-decreasing packing; exception-bearing molecules
    # first-fit into bins 0..EXBINS-1 only
    per_core = []
    nb_max = 0
    for c in range(NCORES):
        stats = []
        for m in range(BL):
            bm = c * BL + m
            sel = np.nonzero(main_mask[bm])[0]
            exc = np.nonzero(exc_mask[bm])[0]
            if len(sel) + len(exc) == 0:
                continue
            uniq = set()
            for a in list(sel) + list(exc):
                uniq.add(int(a))
                for e in edges[bm, a]:
                    if e >= 0:
                        uniq.add(int(e))
            if len(uniq) > A or len(sel) > SLOTB or len(exc) > EXB:
                return None
            stats.append((m, len(uniq), len(sel), len(exc)))
        bins = []          # list of [rows, slots, excs, mols]
        ok = [True]

        def fit(m, nr, nsl, nex, hi):
            for bi in range(min(hi, len(bins))):
                r, s, e, mols = bins[bi]
                if r + nr <= A and s + nsl <= SLOTB and e + nex <= EXB:
                    bins[bi] = [r + nr, s + nsl, e + nex, mols + [m]]
                    return
            if len(bins) < hi:
                bins.append([nr, nsl, nex, [m]])
                return
            ok[0] = False

        for m, nr, nsl, nex in [t for t in stats if t[3] > 0]:
            fit(m, nr, nsl, nex, EXBINS)
        for m, nr, nsl, nex in sorted([t for t in stats if t[3] == 0],
                                      key=lambda t: -t[1]):
            fit(m, nr, nsl, nex, NBMAX)
        if not ok[0]:
            return None
        per_core.append([b[3] for b in bins])
        nb_max = max(nb_max, len(bins))
    NB = nb_max
    MAIN = NB * SLOTB

    et = np.zeros((NCORES, A, NB * KB), dtype=np.float32)
    gidx = np.zeros((NCORES, NB, A), dtype=np.int64)
    main_rows = [[] for _ in range(NCORES)]   # (slot, mol, atom)
    exc_rows = [[] for _ in range(NCORES)]    # (eslot, mol, atom)
    bidx_m = np.zeros((NCORES, MAIN), dtype=np.int64)
    bval_m = np.zeros((NCORES, MAIN), dtype=bool)
    bidx_e = np.zeros((NCORES, EXCU), dtype=np.int64)
    bval_e = np.zeros((NCORES, EXCU), dtype=bool)
    excdeg = {}

    for c in range(NCORES):
        for bnum, mols in enumerate(per_core[c]):
            off = 0            # bin-local gathered-atom row
            k = 0              # main slot within bin
            ke = 0             # exc slot within bin
            loc = {}
            uniq = []
            for m in mols:
                bm = c * BL + m
                sel = np.nonzero(main_mask[bm])[0]
                exc = np.nonzero(exc_mask[bm])[0]
                for a in list(sel) + list(exc):
                    srcs = [int(a)] + [int(e) for e in edges[bm, a] if e >= 0]
                    for s2 in srcs:
                        key = (m, s2)
                        if key not in loc:
                            loc[key] = len(uniq)
                            uniq.append(key)
                    if deg[bm, a] == D - 1:
                        col = bnum * KB + k
                        slot = bnum * SLOTB + k
                        k += 1
                        main_rows[c].append((slot, m, int(a)))
                        bidx_m[c, slot] = m * A + int(a)
                        bval_m[c, slot] = True
                    else:
                        col = bnum * KB + SLOTB + ke
                        eslot = bnum * EXB + ke
                        ke += 1
                        exc_rows[c].append((eslot, m, int(a)))
                        bidx_e[c, eslot] = m * A + int(a)
                        bval_e[c, eslot] = True
                        excdeg[(c, eslot)] = int(deg[bm, a])
                    for s2 in srcs:
                        et[c, loc[(m, s2)], col] += 1.0
            assert len(uniq) <= A
            for r, (m, s2) in enumerate(uniq):
                gidx[c, bnum, r] = m * A + s2

    dlist = sorted(set(excdeg.values()))
    L = len(dlist)
    LW = max(L, 1)
    md = np.zeros((NCORES, A, LW * EXCU), dtype=np.float32)
    for (c, eslot), d in excdeg.items():
        md[c, :, dlist.index(d) * EXCU + eslot] = 1.0
    return {
        "NB": NB,
        "dlist": dlist,
        "et": et.astype(bf),
        "md": md.astype(bf),
        "gidx": gidx,
        "main_rows": main_rows,
        "exc_rows": exc_rows,
        "bidx_m": bidx_m,
        "bval_m": bval_m,
        "bidx_e": bidx_e,
        "bval_e": bval_e,
    }


def _make_in_maps_sparse(atoms, bonds, W, b, meta):
    import ml_dtypes

    bf = ml_dtypes.bfloat16
    NB = meta["NB"]
    MAIN = NB * SLOTB
    PF = NB * (SLOTB + EXB)
    L = len(meta["dlist"])
    LW = max(L, 1)
    LC = LW * C
    AUXC = D * EXCU + LW * EXCU

    atoms_flat = atoms.reshape(NCORES, BL * A, FA)
    # gathered atoms, partition-major: [NCORES, A, NB, FA]
    atoms8 = atoms_flat[np.arange(NCORES)[:, None, None],
                        meta["gidx"]].transpose(0, 2, 1, 3)

    waug = np.concatenate([W, b[:, None, :]], axis=1)     # (5, 321, 256)
    # ch-major: [ch half][f chunk] of 128 cols each
    wd4p = np.zeros((128, 3 * C), dtype=np.float32)
    for ch in range(2):
        cs = ch * 128
        wd4p[:, ch * 384:ch * 384 + 128] = waug[D - 1][0:128, cs:cs + 128]
        wd4p[:, ch * 384 + 128:ch * 384 + 256] = \
            waug[D - 1][128:256, cs:cs + 128]
        wd4p[0:FAUG - 256, ch * 384 + 256:ch * 384 + 384] = \
            waug[D - 1][256:FAUG, cs:cs + 128]
    wexc = np.zeros((3, 128, LC), dtype=np.float32)       # per f-chunk
    for i, d in enumerate(meta["dlist"]):
        wexc[0, :, i * C:(i + 1) * C] = waug[d][0:128]
        wexc[1, :, i * C:(i + 1) * C] = waug[d][128:256]
        wexc[2, 0:FAUG - 256, i * C:(i + 1) * C] = waug[d][256:FAUG]
    wd4p = wd4p.astype(bf)
    wexc = wexc.astype(bf)

    bonds_flat = bonds.reshape(NCORES, BL * A, D, FB)
    in_maps = []
    for c in range(NCORES):
        bm = bonds_flat[c][meta["bidx_m"][c]]             # (MAIN, D, FB)
        bm = bm * meta["bval_m"][c][:, None, None]
        be = bonds_flat[c][meta["bidx_e"][c]]             # (EXCU, D, FB)
        be = be * meta["bval_e"][c][:, None, None]
        bondm = np.ascontiguousarray(
            bm.transpose(2, 1, 0).reshape(FB, D * MAIN)).astype(bf)
        auxp = np.zeros((A, AUXC), dtype=np.float32)
        auxp[0:FB, 0:D * EXCU] = be.transpose(2, 1, 0).reshape(FB, D * EXCU)
        auxp[:, D * EXCU:AUXC] = meta["md"][c]

        s1 = np.zeros((A, PF + 2 * FA), dtype=np.float32)
        s1[:, 0:PF] = meta["et"][c]
        s1[:, PF:PF + FA] = atoms8[c, :, 0]
        s1[:, PF + FA:PF + 2 * FA] = atoms8[c, :, 1]
        s2 = atoms8[c, :, 4:NB].reshape(A, (NB - 4) * FA)
        c2 = np.concatenate([atoms8[c, :, 2], atoms8[c, :, 3]], axis=1)
        im = {
            "s1": s1.astype(bf),
            "s2": np.ascontiguousarray(s2).astype(bf),  # bins 4..
            "c1": wd4p,
            "c2": np.ascontiguousarray(c2).astype(bf),
            "bondm": bondm,
            "aux": auxp.astype(bf),
            "wexc": wexc,
        }
        in_maps.append(im)
    return in_maps


# ---------------------------------------------------------------------------
# dense fallback program (unchanged from the baseline kernel)
# ---------------------------------------------------------------------------

def _build_program_dense():
    from contextlib import ExitStack

    import concourse.bass as bass
    import concourse.tile as tile
    from concourse import bacc, mybir

    f32 = mybir.dt.float32
    AF = mybir.ActivationFunctionType
    OP = mybir.AluOpType
    f32r = mybir.dt.float32r
    bf16 = mybir.dt.bfloat16

    nc = bacc.Bacc("TRN2", target_bir_lowering=False, debug=False,
                   num_devices=NCORES)

    atoms_d = nc.dram_tensor("atoms", [BL, A, FA], f32r, kind="ExternalInput")
    bonds_d = nc.dram_tensor("bonds", [BL, A, D * FB], f32,
                             kind="ExternalInput")
    edges_d = nc.dram_tensor("edges", [BL, A, A * D], bf16,
                             kind="ExternalInput")
    waug_d = nc.dram_tensor("waug", [D, FAUG, C], f32r, kind="ExternalInput")
    ident_d = nc.dram_tensor("ident", [A, A], f32, kind="ExternalInput")
    identr_d = nc.dram_tensor("identr", [A, A], f32r, kind="ExternalInput")
    identb_d = nc.dram_tensor("identb", [A, A], bf16, kind="ExternalInput")
    iota_d = nc.dram_tensor("iota", [A, 1], f32, kind="ExternalInput")
    edeg_d = nc.dram_tensor("edeg", [BL, A, D], f32, kind="ExternalInput")
    onesr_d = nc.dram_tensor("onesr", [1, A], f32, kind="ExternalInput")
    out_d = nc.dram_tensor("out", [BL, A, C], f32, kind="ExternalOutput")

    atoms_ap = atoms_d.ap()
    bonds_ap = bonds_d.ap()
    edges_ap = edges_d.ap()
    out_ap = out_d.ap()

    with tile.TileContext(nc) as tc, ExitStack() as ctx:
        consts = ctx.enter_context(tc.tile_pool(name="consts", bufs=1))
        pin = ctx.enter_context(tc.tile_pool(name="pin", bufs=3))
        pbc = ctx.enter_context(tc.tile_pool(name="pbc", bufs=2))
        pet = ctx.enter_context(tc.tile_pool(name="pet", bufs=2))
        pfeat = ctx.enter_context(tc.tile_pool(name="pfeat", bufs=2))
        pmd = ctx.enter_context(tc.tile_pool(name="pmd", bufs=2))
        pz = ctx.enter_context(tc.tile_pool(name="pz", bufs=2))
        pout = ctx.enter_context(tc.tile_pool(name="pout", bufs=3))
        ps_f = ctx.enter_context(
            tc.tile_pool(name="ps_f", bufs=2, space="PSUM"))
        ps_c2 = ctx.enter_context(
            tc.tile_pool(name="ps_c2", bufs=1, space="PSUM"))
        ps_z = ctx.enter_context(
            tc.tile_pool(name="ps_z", bufs=1, space="PSUM"))
        ps_s = ctx.enter_context(
            tc.tile_pool(name="ps_s", bufs=1, space="PSUM"))

        G4 = 4
        ident = consts.tile([A, A], f32)
        nc.scalar.dma_start(out=ident[:], in_=ident_d.ap()[:])
        identr = consts.tile([A, A], f32r)
        nc.scalar.dma_start(out=identr[:], in_=identr_d.ap()[:])
        iota_col = consts.tile([A, 1], f32)
        nc.gpsimd.dma_start(out=iota_col[:], in_=iota_d.ap()[:])
        ones_row = consts.tile([1, A], f32)
        nc.scalar.dma_start(out=ones_row[:], in_=onesr_d.ap()[:])
        identb4 = consts.tile([A, G4 * A], bf16)
        for j in range(G4):
            nc.gpsimd.dma_start(out=identb4[:, j * A:(j + 1) * A],
                                in_=identb_d.ap()[:])

        w0 = consts.tile([128, D * C], f32r)
        w1 = consts.tile([128, D * C], f32r)
        w2 = consts.tile([FAUG - 256, D * C], f32r)
        for d in range(D):
            nc.scalar.dma_start(out=w0[:, d * C:(d + 1) * C],
                                in_=waug_d.ap()[d, 0:128, :])
            nc.scalar.dma_start(out=w1[:, d * C:(d + 1) * C],
                                in_=waug_d.ap()[d, 128:256, :])
            nc.scalar.dma_start(out=w2[:, d * C:(d + 1) * C],
                                in_=waug_d.ap()[d, 256:FAUG, :])

        for bg in range(BL // G4):
            mols = range(bg * G4, (bg + 1) * G4)
            atoms4 = pin.tile([A, G4 * FA], f32r)
            nc.sync.dma_start(
                out=atoms4.rearrange("p (g f) -> p g f", g=G4),
                in_=atoms_ap[bg * G4:(bg + 1) * G4].rearrange(
                    "g p f -> p g f"))
            bonds4 = pin.tile([A, G4 * D * FB], f32)
            nc.sync.dma_start(
                out=bonds4.rearrange("p (g f) -> p g f", g=G4),
                in_=bonds_ap[bg * G4:(bg + 1) * G4].rearrange(
                    "g p f -> p g f"))
            bc_e4 = pbc.tile([A, G4 * A * D], bf16)
            nc.gpsimd.dma_start(
                out=bc_e4.rearrange("p (g f) -> p g f", g=G4),
                in_=edges_ap[bg * G4:(bg + 1) * G4].rearrange(
                    "g p f -> p g f"))
            edeg4 = pfeat.tile([A, G4 * D], f32)
            nc.sync.dma_start(
                out=edeg4.rearrange("p (g f) -> p g f", g=G4),
                in_=edeg_d.ap()[bg * G4:(bg + 1) * G4].rearrange(
                    "g p f -> p g f"))
            ne4 = pfeat.tile([A, G4 * D], f32)
            nc.vector.tensor_scalar(ne4[:], edeg4[:], -1.0, None,
                                    OP.not_equal)
            degp1_4 = pfeat.tile([A, G4], f32)
            nc.vector.tensor_reduce(
                degp1_4[:], ne4.rearrange("p (g d) -> p g d", g=G4),
                axis=mybir.AxisListType.X, op=OP.add)
            nc.vector.tensor_scalar(degp1_4[:], degp1_4[:], 1.0, None,
                                    OP.add)

            cmp5 = pbc.tile([A, G4 * A * D], bf16)
            nc.vector.tensor_scalar(cmp5[:], bc_e4[:], iota_col[:], None,
                                    OP.is_equal)
            cg = cmp5.rearrange("p (g d a) -> p g d a", g=G4, d=D)
            t01 = pet.tile([A, G4 * A], bf16)
            nc.vector.tensor_add(t01[:], cg[:, :, 0, :], cg[:, :, 1, :])
            t23 = pet.tile([A, G4 * A], bf16)
            nc.vector.tensor_add(t23[:], cg[:, :, 2, :], cg[:, :, 3, :])
            t4i = pet.tile([A, G4 * A], bf16)
            nc.vector.tensor_add(t4i[:], cg[:, :, 4, :], identb4[:])
            t0123 = pet.tile([A, G4 * A], bf16)
            nc.vector.tensor_add(t0123[:], t01[:], t23[:])
            etp4 = pet.tile([A, G4 * A], f32r)
            with nc.allow_low_precision(reason="exact small-int counts"):
                nc.vector.tensor_add(etp4[:], t0123[:], t4i[:])

            out4 = pout.tile([A, G4 * C], f32)
            for j, bm in enumerate(mols):
                etp = etp4[:, j * A:(j + 1) * A]
                atoms_sb = atoms4[:, j * FA:(j + 1) * FA]
                bonds_sb = bonds4[:, j * D * FB:(j + 1) * D * FB]

                degp1 = degp1_4[:, j:j + 1]

                pf = ps_f.tile([A, FA], f32)
                nc.tensor.matmul(pf[:, 0:128], atoms_sb[:, 0:128], etp)
                nc.tensor.matmul(pf[:, 128:256], atoms_sb[:, 128:256], etp)

                featT01 = pfeat.tile([A, FA], f32r)
                nc.scalar.copy(featT01[:], pf[:, 0:FA])

                sumbond = pfeat.tile([A, FB], f32r)
                with nc.allow_low_precision(
                        reason="f32r rounding of bond sums"):
                    nc.vector.reduce_sum(
                        sumbond[:],
                        bonds_sb.rearrange("p (d f) -> p f d", d=D),
                        axis=mybir.AxisListType.X)
                pc2 = ps_c2.tile([FB, A], f32)
                nc.tensor.matmul(pc2[:], sumbond[:], identr[:])
                chunk2 = pfeat.tile([FAUG - 256, A], f32r)
                nc.scalar.copy(chunk2[0:FB, :], pc2[:])
                nc.vector.tensor_copy(chunk2[FB:FB + 1, :], ones_row[:])

                md = pmd.tile([A, D * A], f32r)
                for d in range(D):
                    nc.vector.tensor_scalar(md[:, d * A:(d + 1) * A],
                                            ident[:], degp1[:], float(d + 1),
                                            OP.mult, OP.is_equal)

                lhs = [featT01[:, 0:128], featT01[:, 128:256], chunk2[:]]
                rhs = [w0, w1, w2]
                groups = [(0, 512), (512, 1024), (1024, 1280)]
                zsb = pz.tile([A, D * C], f32r)
                for g0, g1 in groups:
                    pzg = ps_z.tile([A, 512], f32, tag="pzg", bufs=4)
                    nc.tensor.matmul(pzg[:, 0:g1 - g0], lhs[0],
                                     rhs[0][:, g0:g1], start=True, stop=False)
                    nc.tensor.matmul(pzg[:, 0:g1 - g0], lhs[1],
                                     rhs[1][:, g0:g1], start=False,
                                     stop=False)
                    nc.tensor.matmul(pzg[:, 0:g1 - g0], lhs[2],
                                     rhs[2][:, g0:g1], start=False, stop=True)
                    nc.scalar.copy(zsb[:, g0:g1], pzg[:, 0:g1 - g0])

                pst = ps_s.tile([A, C], f32)
                for d in range(D):
                    nc.tensor.matmul(pst[:], md[:, d * A:(d + 1) * A],
                                     zsb[:, d * C:(d + 1) * C],
                                     start=(d == 0), stop=(d == D - 1))
                nc.scalar.activation(out4[:, j * C:(j + 1) * C], pst[:],
                                     AF.Relu)
            nc.gpsimd.dma_start(
                out=out_ap[bg * G4:(bg + 1) * G4].rearrange("g p f -> p g f"),
                in_=out4.rearrange("p (g f) -> p g f", g=G4))

    nc.compile()
    return nc


def _make_in_maps_dense(atoms, bonds, edges, W, b):
    atoms = np.ascontiguousarray(np.asarray(atoms, dtype=np.float32))
    bonds = np.ascontiguousarray(np.asarray(bonds, dtype=np.float32))
    edges = np.asarray(edges)
    W = np.asarray(W, dtype=np.float32)
    b = np.asarray(b, dtype=np.float32)

    import ml_dtypes
    edges_f = np.ascontiguousarray(edges.transpose(0, 2, 1)).reshape(
        B, D * A).astype(ml_dtypes.bfloat16)
    edges_rep = np.ascontiguousarray(
        np.broadcast_to(edges_f[:, None, :], (B, A, D * A)))

    waug = np.ascontiguousarray(
        np.concatenate([W, b[:, None, :]], axis=1))           # (5, 321, 256)
    ident = np.eye(A, dtype=np.float32)
    iota = np.arange(A, dtype=np.float32).reshape(A, 1)
    onesr = np.ones((1, A), dtype=np.float32)

    edeg8 = edges.reshape(NCORES, BL, A, D).astype(np.float32)
    atoms8 = atoms.reshape(NCORES, BL, A, FA)
    bonds8 = bonds.reshape(NCORES, BL, A, D * FB)
    edges8 = edges_rep.reshape(NCORES, BL, A, A * D)

    return [
        {
            "atoms": atoms8[c],
            "bonds": bonds8[c],
            "edges": edges8[c],
            "waug": waug,
            "ident": ident,
            "identr": ident,
            "identb": ident.astype(ml_dtypes.bfloat16),
            "iota": iota,
            "edeg": edeg8[c],
            "onesr": onesr,
        }
        for c in range(NCORES)
    ]


# ---------------------------------------------------------------------------
# entry points
# ---------------------------------------------------------------------------

def run_sharded(atoms, bonds, edges, W, b, trace=False):
    """Run on the 8 NeuronCores; returns (output, BassKernelResults)."""
    from concourse.bass_utils import run_bass_kernel_spmd

    atoms = np.ascontiguousarray(np.asarray(atoms, dtype=np.float32))
    bonds = np.ascontiguousarray(np.asarray(bonds, dtype=np.float32))
    edges = np.asarray(edges)
    W = np.asarray(W, dtype=np.float32)
    b = np.asarray(b, dtype=np.float32)

    meta = _sparse_metadata(edges)
    if meta is None:
        if "dense" not in _CACHE:
            _CACHE["dense"] = _build_program_dense()
        nc = _CACHE["dense"]
        in_maps = _make_in_maps_dense(atoms, bonds, edges, W, b)
        res = run_bass_kernel_spmd(nc, in_maps, list(range(NCORES)),
                                   trace=trace)
        out = np.concatenate(
            [res.results[c]["out"] for c in range(NCORES)],
            axis=0).reshape(B, A, C)
        return out, res

    L = len(meta["dlist"])
    NB = meta["NB"]
    key = ("sparse", L, NB)
    if key not in _CACHE:
        _CACHE[key] = _build_program_sparse(L, NB)
    nc = _CACHE[key]
    in_maps = _make_in_maps_sparse(atoms, bonds, W, b, meta)
    res = run_bass_kernel_spmd(nc, in_maps, list(range(NCORES)), trace=trace)

    MAIN = NB * SLOTB
    out = np.zeros((B, A, C), dtype=np.float32)
    for c in range(NCORES):
        # out is transposed on device: [2, 128, MAIN], C row = ch*128+r
        zt = np.asarray(res.results[c]["out"],
                        dtype=np.float32).reshape(2 * 128, MAIN)
        main = zt.T                                      # [MAIN, C]
        exc = np.asarray(res.results[c]["oute"], dtype=np.float32)
        for slot, ml, a in meta["main_rows"][c]:
            out[c * BL + ml, a] = main[slot]
        for eslot, ml, a in meta["exc_rows"][c]:
            out[c * BL + ml, a] = exc[eslot]
    return out, res


def kernel(atoms, bonds, edges, W, b):
    out, _ = run_sharded(atoms, bonds, edges, W, b)
    return out
